# revision 14
# baseline (speedup 1.0000x reference)
"""BEV cross-attention kernel for Trainium2, 8-core SPMD.

Shard: core c handles (batch b=c//4, query slice r=c%4 of 256 BEV queries),
computing ALL 4 heads for its queries. Keys/values (6 cams x 1680) are
replicated per core. No collectives: each core's output is a disjoint
[D, 256] token slice; the host concatenates.

Layout: feature-major ("S^T") attention - scores [keys=120p, (head, q)=1024f]
so softmax exp runs on ScalarE with per-partition (per-key) scale=rstd_k and
bias=ln(rstd_v) (K/V LayerNorms folded through the exp; shared by all heads).
LN means fold into centered projection weights host-side; the softmax
denominator rides the PV matmul as a per-head ones column of V. No max
subtraction (logits are small by construction).

Engine budget: ScalarE does the 84 exps (the wall, ~88us); PE does all
projections + QK/PV in bf16 (1 cyc/col); DVE does squares (bf16 2x) and
evacs; Pool does V evacs, cross-partition reduces and broadcasts. Per-token
LN stats are produced token-major directly by 1-col PE matmuls against a
ones vector (no DRAM bounces anywhere).
"""
import numpy as np
import ml_dtypes

import concourse.bass as bass
import concourse.bass_isa as bass_isa
import concourse.mybir as mybir
import concourse.tile as tile
from concourse.bass_utils import run_bass_kernel_spmd

F32 = mybir.dt.float32
F32R = mybir.dt.float32r
BF16 = mybir.dt.bfloat16

HEADS, DH, D = 4, 32, 128
B, NCAM = 2, 6
Q = 32 * 32            # 1024 BEV queries per batch
QS = Q // 4            # 256 queries per core
KC = 28 * 60           # 1680 keys per camera
CW = 120               # key chunk width: 1680 = 14 * 120, no tail
NKCH = KC // CW        # 14
N_CORES = 8
EPS = 1e-5
SCALE = DH ** -0.5

_cached = {}


# ---------------------------------------------------------------------------
# walrus compat: this container's walrus rejects instructions carrying more
# than one semaphore wait; move excess waits onto same-engine NoOps.
_COMPUTE_ENGINES = None
_nopctr = [0]


def _split_sync_waits(nc, limit=1):
    global _COMPUTE_ENGINES
    if _COMPUTE_ENGINES is None:
        _COMPUTE_ENGINES = {
            mybir.EngineType.PE, mybir.EngineType.Activation,
            mybir.EngineType.Pool, mybir.EngineType.DVE, mybir.EngineType.SP,
        }
    for f in nc.m.functions:
        for bb in f.blocks:
            out, changed = [], False
            for inst in bb.instructions:
                si = inst.sync_info
                if (si is not None and len(si.on_wait) > limit
                        and inst.engine in _COMPUTE_ENGINES):
                    waits = list(si.on_wait)
                    n_extra = len(waits) - limit
                    for i in range(0, n_extra, limit):
                        nop = mybir.InstNoOp(name=f"wait-split-{_nopctr[0]}")
                        _nopctr[0] += 1
                        nop.engine = inst.engine
                        nop.sync_info = mybir.SyncInfo(
                            on_wait=waits[i:min(i + limit, n_extra)], on_update=[])
                        out.append(nop)
                    si.on_wait = waits[n_extra:]
                    changed = True
                out.append(inst)
            if changed:
                bb.instructions = out


# ---------------------------------------------------------------------------
def _build_program(split=True, collective=True, n_dev=N_CORES):
    nc = bass.Bass("TRN2", target_bir_lowering=False, debug=False,
                   num_devices=n_dev)

    xq = nc.dram_tensor("xq", [NCAM, D, QS], BF16, kind="ExternalInput").ap()
    xk = nc.dram_tensor("xk", [NCAM, D, KC], BF16, kind="ExternalInput").ap()
    xv = nc.dram_tensor("xv", [NCAM, D, KC], BF16, kind="ExternalInput").ap()
    # packed bf16 weights: [wq 128 | wk 128 | wv_ext 132 | w1 256 | w2 256]
    wcat = nc.dram_tensor("wcat", [D, 900], BF16, kind="ExternalInput").ap()
    wp = nc.dram_tensor("wp", [DH, HEADS, D], F32R, kind="ExternalInput").ap()
    # packed f32 per-feature consts:
    # [wbq_pairA, wbq_pairB, bp', b1_0, b1_1, b2, pre_g, pre_b, post_g,
    #  post_b] (wbq pair columns hold heads 0-1 / 2-3 in partitions 0..63)
    fcon = nc.dram_tensor("fcon", [D, 10], F32, kind="ExternalInput").ap()
    skipb = nc.dram_tensor("skipb", [D, QS], F32, kind="ExternalInput").ap()

    out = nc.dram_tensor("out", [D, QS], F32, kind="ExternalOutput").ap()

    EXP = mybir.ActivationFunctionType.Exp
    LN_ = mybir.ActivationFunctionType.Ln
    SQRT = mybir.ActivationFunctionType.Sqrt
    GELU = mybir.ActivationFunctionType.Gelu
    ADD = mybir.AluOpType.add
    MULT = mybir.AluOpType.mult

    with tile.TileContext(nc) as tc:
        with tc.tile_pool(name="consts", bufs=1) as consts, \
             tc.tile_pool(name="loads", bufs=3) as loads, \
             tc.tile_pool(name="sq", bufs=2) as sqp, \
             tc.tile_pool(name="kv", bufs=3) as kvp, \
             tc.tile_pool(name="sml", bufs=2) as sml, \
             tc.tile_pool(name="ee", bufs=3) as eep, \
             tc.tile_pool(name="fin", bufs=1) as finp:

            # ---- constant tiles (DMAs are issued after the first
            # camera loads, in the schedule section) ----
            wcat_t = consts.tile([D, 900], BF16, name="wcat_t")
            wq_t = wcat_t[:, 0:128]
            wk_t = wcat_t[:, 128:256]
            wv_t = wcat_t[:, 256:388]          # [D, 4*33]
            w1_t = wcat_t[:, 388:644]
            w2_t = wcat_t[:, 644:900].rearrange("p (f d) -> p f d", f=2)
            wp_t = consts.tile([DH, HEADS, D], F32R, name="wp_t")
            fcon_t = consts.tile([D, 10], F32, name="fcon_t")
            wbq_ab = fcon_t[:, 0:2]
            bpp_t = fcon_t[:, 2:3]
            b1_t = fcon_t[:, 3:5]
            b2_t = fcon_t[:, 5:6]
            preg_t = fcon_t[:, 6:7]
            preb_t = fcon_t[:, 7:8]
            postg_t = fcon_t[:, 8:9]
            postb_t = fcon_t[:, 9:10]
            skip_t = consts.tile([D, QS], F32, name="skip_t")

            eps_t = consts.tile([D, 1], F32, name="eps_t")
            nc.vector.memset(eps_t, EPS)
            onesb = consts.tile([D, 2], BF16, name="onesb")  # [1 | 1/128]
            nc.vector.memset(onesb[:, 0:1], 1.0)
            nc.vector.memset(onesb[:, 1:2], 1.0 / 128.0)
            onesr = consts.tile([D, 2], F32R, name="onesr")  # [1 | 1/128]
            nc.vector.memset(onesr[:, 0:1], 1.0)
            nc.vector.memset(onesr[:, 1:2], 1.0 / 128.0)
            ones_row = consts.tile([1, D], F32R, name="ones_row")
            nc.vector.memset(ones_row, 1.0)

            # ---- PSUM pools ----
            # banks: avt 2 + sc 2x2 + kp 1 + shared proj 1 = 8
            accpool = tc.tile_pool(name="accp", bufs=1, space="PSUM")
            accp = accpool.__enter__()
            scpool = tc.tile_pool(name="scp", bufs=2, space="PSUM")
            scp = scpool.__enter__()
            projpool = tc.tile_pool(name="projp", bufs=1, space="PSUM")
            projp = projpool.__enter__()

            avt = accp.tile([33, HEADS, QS], F32, name="avt")      # 2 banks
            # bank A: kproj ping-pong [0:210|210:420] | stats [420:476]
            ka_ps = projp.tile([D, 512], F32, name="ka_ps")        # 1 bank
            kp_ps = [ka_ps[0:64, 0:210], ka_ps[0:64, 210:420]]
            st_ps = ka_ps[0:CW, 420:476].rearrange("p (j c) -> p j c", j=4)
            # bank B: vproj ping-pong [0:132|132:264] | qproj [256:512]
            sh_ps = projp.tile([D, 512], F32, name="sh_ps")        # 1 bank
            vp_ps = [sh_ps[0:CW, 0:132], sh_ps[0:CW, 132:264]]
            qp_ps = [sh_ps[0:64, 0:256], sh_ps[0:64, 256:512]]

            # ---- per-camera phase 1, split so the ScalarE ops (finish)
            # can be emitted mid-attention of the previous camera and never
            # block the exp stream ----
            def load(n):
                xk_t = loads.tile([D, KC], BF16, name="xk_t", tag="xk")
                nc.sync.dma_start(out=xk_t, in_=xk[n])
                xv_t = loads.tile([D, KC], BF16, name="xv_t", tag="xv")
                nc.sync.dma_start(out=xv_t, in_=xv[n])
                xq_t = loads.tile([D, QS], BF16, name="xq_t", tag="xq")
                nc.sync.dma_start(out=xq_t, in_=xq[n])
                return xk_t, xv_t, xq_t

            def produce(n, ld):
                xk_t, xv_t, xq_t = ld

                x2k = sqp.tile([D, KC], BF16, name="x2k", tag="x2k")
                nc.vector.tensor_mul(out=x2k, in0=xk_t, in1=xk_t)
                x2v = sqp.tile([D, KC], BF16, name="x2v", tag="x2v")
                nc.vector.tensor_mul(out=x2v, in0=xv_t, in1=xv_t)
                x2q = sqp.tile([D, QS], BF16, name="x2q", tag="x2q")
                nc.vector.tensor_mul(out=x2q, in0=xq_t, in1=xq_t)

                # token-major stats via 1-col matmuls:
                # st rows: 0=k-mean, 1=v-mean, 2=k-sumsq, 3=v-sumsq
                for c in range(NKCH):
                    xkc = xk_t[:, c * CW:(c + 1) * CW]
                    xvc = xv_t[:, c * CW:(c + 1) * CW]
                    x2kc = x2k[:, c * CW:(c + 1) * CW]
                    x2vc = x2v[:, c * CW:(c + 1) * CW]
                    nc.tensor.matmul(st_ps[:, 0, c:c + 1], lhsT=xkc,
                                     rhs=onesb[:, 1:2], start=True, stop=True)
                    nc.tensor.matmul(st_ps[:, 1, c:c + 1], lhsT=xvc,
                                     rhs=onesb[:, 1:2], start=True, stop=True)
                    nc.tensor.matmul(st_ps[:, 2, c:c + 1], lhsT=x2kc,
                                     rhs=onesb[:, 0:1], start=True, stop=True)
                    nc.tensor.matmul(st_ps[:, 3, c:c + 1], lhsT=x2vc,
                                     rhs=onesb[:, 0:1], start=True, stop=True)

                st_sb = sml.tile([CW, 4, NKCH], F32, name="st_sb", tag="st")
                nc.vector.tensor_copy(out=st_sb, in_=st_ps)
                mu2 = sml.tile([CW, 2, NKCH], F32, name="mu2", tag="mu2")
                nc.vector.tensor_mul(out=mu2, in0=st_sb[:, 0:2, :],
                                     in1=st_sb[:, 0:2, :])
                var2 = sml.tile([CW, 2, NKCH], F32, name="var2", tag="var2")
                nc.vector.tensor_scalar_mul(out=var2, in0=st_sb[:, 2:4, :],
                                            scalar1=1.0 / 128.0)
                nc.vector.tensor_sub(out=var2, in0=var2, in1=mu2)

                # q stats rows (Pool C-reduce, off the PE/Act path)
                musum = sml.tile([1, QS], F32, name="musum", tag="musum")
                nc.gpsimd.tensor_reduce(out=musum, in_=xq_t,
                                        axis=mybir.AxisListType.C, op=ADD)
                sssum = sml.tile([1, QS], F32, name="sssum", tag="sssum")
                nc.gpsimd.tensor_reduce(out=sssum, in_=x2q,
                                        axis=mybir.AxisListType.C, op=ADD)
                muq = sml.tile([1, QS], F32, name="muq", tag="muq")
                nc.vector.tensor_scalar_mul(out=muq, in0=musum,
                                            scalar1=1.0 / 128.0)
                mu2q = sml.tile([1, QS], F32, name="mu2q", tag="mu2q")
                nc.vector.tensor_mul(out=mu2q, in0=muq, in1=muq)
                varq = sml.tile([1, QS], F32, name="varq", tag="varq")
                nc.vector.tensor_scalar_mul(out=varq, in0=sssum,
                                            scalar1=1.0 / 128.0)
                nc.vector.tensor_sub(out=varq, in0=varq, in1=mu2q)

                # K projection, feature-major, split in head pairs so
                # attention lhsT slices sit at base partition 0/32
                khT = [kvp.tile([64, KC], BF16, name=f"khT{p}",
                                tag=f"khT{p}") for p in range(2)]
                for i, (j, p) in enumerate(
                        (j, p) for j in range(8) for p in range(2)):
                    kp = kp_ps[i % 2]
                    nc.tensor.matmul(
                        kp, lhsT=wk_t[:, p * 64:(p + 1) * 64],
                        rhs=xk_t[:, j * 210:(j + 1) * 210],
                        start=True, stop=True)
                    eng = nc.vector if i % 2 == 0 else nc.gpsimd
                    eng.tensor_copy(
                        out=khT[p][:, j * 210:(j + 1) * 210], in_=kp)

                # V projection, token-major [120, 4, 33] per chunk
                # (col 32 of each head block is 0 from wv_ext; memset to 1
                # afterwards: softmax denominator ride-along)
                vhE = kvp.tile([CW, NKCH, HEADS, 33], BF16, name="vhE",
                               tag="vhE")
                nc.gpsimd.memset(vhE[:, :, :, 32], 1.0)
                for c in range(NKCH):
                    xvc = xv_t[:, c * CW:(c + 1) * CW]
                    vp = vp_ps[c % 2]
                    nc.tensor.matmul(vp, lhsT=xvc, rhs=wv_t,
                                     start=True, stop=True)
                    nc.gpsimd.tensor_copy(
                        out=vhE[:, c, :, 0:32],
                        in_=vp.rearrange("p (h d) -> p h d", h=4)[:, :, 0:32])
                return xq_t, var2, varq, khT, vhE

            def finish(n, prod):
                xq_t, var2, varq, khT, vhE = prod
                # ln(var+eps) for both K and V in one ScalarE op
                lnb = sml.tile([CW, 2, NKCH], F32, name="lnb", tag="lnb")
                nc.scalar.activation(out=lnb, in_=var2, func=LN_,
                                     bias=eps_t[0:CW, :], scale=1.0)
                rstdk = kvp.tile([CW, NKCH], F32, name="rstdk", tag="rstdk")
                nc.scalar.activation(out=rstdk, in_=lnb[:, 0, :], func=EXP,
                                     bias=0.0, scale=-0.5)
                lnrv = kvp.tile([CW, NKCH], F32, name="lnrv", tag="lnrv")
                nc.vector.tensor_scalar_mul(out=lnrv, in0=lnb[:, 1, :],
                                            scalar1=-0.5)

                sdq = sml.tile([1, QS], F32, name="sdq", tag="sdq")
                nc.scalar.activation(out=sdq, in_=varq, func=SQRT,
                                     bias=eps_t[0:1, :], scale=1.0)
                rqrow = sml.tile([1, QS], F32, name="rqrow", tag="rqrow")
                nc.vector.reciprocal(out=rqrow, in_=sdq)
                rqbc = sml.tile([D, QS], F32, name="rqbc", tag="rqbc")
                nc.gpsimd.partition_broadcast(rqbc, rqrow, channels=D)
                # pre-scale x by rstd_q (commutes with the centered
                # projection), so the q matmuls have no ScalarE dependency
                xqn = sml.tile([D, QS], BF16, name="xqn", tag="xqn")
                nc.vector.tensor_mul(out=xqn, in0=xq_t, in1=rqbc)
                qhT = [kvp.tile([64, QS], BF16, name=f"qhT{p}",
                                tag=f"qhT{p}") for p in range(2)]
                for p in range(2):
                    nc.tensor.matmul(qp_ps[p],
                                     lhsT=wq_t[:, p * 64:(p + 1) * 64],
                                     rhs=xqn, start=True, stop=True)
                    nc.vector.tensor_scalar_add(
                        out=qhT[p], in0=qp_ps[p],
                        scalar1=wbq_ab[0:64, p:p + 1])
                return khT, vhE, rstdk, lnrv, qhT

            # ---- attention for one camera (chunk range) ----
            def attention(n, cam, c0, c1):
                khT, vhE, rstdk, lnrv, qhT = cam
                for c in range(c0, c1):
                    sc_ps = scp.tile([CW, HEADS, QS], F32, name="sc_ps",
                                     tag="sc")
                    for h in range(HEADS):
                        p, hh = divmod(h, 2)
                        nc.tensor.matmul(
                            sc_ps[:, h, :],
                            lhsT=khT[p][hh * DH:(hh + 1) * DH,
                                        c * CW:(c + 1) * CW],
                            rhs=qhT[p][hh * DH:(hh + 1) * DH, :],
                            start=True, stop=True)
                    et = eep.tile([CW, HEADS, QS], BF16, name="et", tag="et")
                    nc.scalar.activation(out=et, in_=sc_ps, func=EXP,
                                         bias=lnrv[:, c:c + 1],
                                         scale=rstdk[:, c:c + 1])
                    first = (n == 0 and c == 0)
                    last = (n == NCAM - 1 and c == NKCH - 1)
                    for h in range(HEADS):
                        nc.tensor.matmul(
                            avt[:, h, :],
                            lhsT=vhE[:, c, h, 0:33],
                            rhs=et[:, h, :],
                            start=first, stop=last)

            # ---- pipelined schedule: produce(n+1) and finish(n+1) are
            # emitted around the first half of attention(n) so no engine's
            # in-order queue ever blocks the exp stream ----
            cams = [None] * NCAM
            prods = [None] * NCAM
            ld0 = load(0)
            nc.sync.dma_start(out=wcat_t, in_=wcat)
            nc.sync.dma_start(out=fcon_t, in_=fcon)
            ld1 = load(1)
            nc.sync.dma_start(out=wp_t, in_=wp)
            nc.sync.dma_start(out=skip_t, in_=skipb)
            prods[0] = produce(0, ld0)
            cams[0] = finish(0, prods[0])
            attention(0, cams[0], 0, 3)
            prods[1] = produce(1, ld1)
            for n in range(NCAM):
                attention(n, cams[n], 3 if n == 0 else 0, 4)
                if n + 1 < NCAM:
                    cams[n + 1] = finish(n + 1, prods[n + 1])
                attention(n, cams[n], 4, 9)
                if n + 2 < NCAM:
                    prods[n + 2] = produce(n + 2, load(n + 2))
                attention(n, cams[n], 9, NKCH)

            # ---- tail: normalize, project, skip, LN, MLP, LN ----
            # The whole tail is per-token, so run two independent 128-token
            # half-chains with interleaved emission: the engines pipeline
            # across halves instead of idling through one serial chain.
            HQ = QS // 2

            projpool.__exit__(None, None, None)
            scpool.__exit__(None, None, None)
            fpool = tc.tile_pool(name="fps", bufs=1, space="PSUM")
            fps = fpool.__enter__()
            rd_ps = [fps.tile([DH, HEADS, HQ], F32, name=f"rd_ps{i}")
                     for i in range(2)]
            zo_ps = fps.tile([D, QS], F32, name="zo_ps", tag="zo")
            row_ps = fps.tile([1, 2, QS], F32, name="row_ps")
            mr_ps = fps.tile([D, 2, QS], F32, name="mr_ps")
            h_ps = fps.tile([D, 2, QS], F32, name="h_ps")

            rden = finp.tile([1, HEADS, QS], F32R, name="rden")
            anorm = finp.tile([DH, HEADS, QS], F32R, name="anorm")
            zt = finp.tile([D, QS], F32R, name="zt")

            def half(t, i):
                """column-half slice of a [..., QS]-shaped AP"""
                return t[..., i * HQ:(i + 1) * HQ]

            steps = []

            def step(f):
                steps.append(f)

            @step
            def _recip(i, cx):
                with nc.allow_low_precision(reason="f32r denominator"):
                    nc.vector.reciprocal(out=half(rden, i),
                                         in_=half(avt[32:33, :, :], i))

            @step
            def _rdbc(i, cx):
                nc.tensor.matmul(
                    rd_ps[i], lhsT=ones_row[:, 0:DH],
                    rhs=half(rden, i), start=True, stop=True)

            @step
            def _anorm(i, cx):
                nc.vector.tensor_mul(out=half(anorm, i),
                                     in0=half(avt[0:32, :, :], i),
                                     in1=rd_ps[i])

            @step
            def _zp(i, cx):
                for h in range(HEADS):
                    nc.tensor.matmul(half(zo_ps, i), lhsT=wp_t[:, h, :],
                                     rhs=half(anorm, i)[:, h, :],
                                     start=(h == 0), stop=(h == HEADS - 1))

            @step
            def _zt(i, cx):
                nc.vector.tensor_add(out=half(zt, i), in0=half(zo_ps, i),
                                     in1=half(skip_t, i))

            @step
            def _ztb(i, cx):
                nc.vector.tensor_scalar_add(out=half(zt, i), in0=half(zt, i),
                                            scalar1=bpp_t)

            def feat_ln_steps(src_t, gain, bias_, dst_t, nm):
                s2 = finp.tile([D, QS], F32R, name=nm + "_s2", tag=nm + "s2")
                murow = sml.tile([1, 2, QS], F32R, name=nm + "_mu",
                                 tag=nm + "mu")
                vr = sml.tile([1, QS], F32, name=nm + "_vr", tag=nm + "vr")
                sd = sml.tile([1, QS], F32, name=nm + "_sd", tag=nm + "sd")
                zc = finp.tile([D, QS], F32R, name=nm + "_zc", tag=nm + "zc")

                @step
                def _sq(i, cx):
                    nc.vector.tensor_mul(out=half(s2, i), in0=half(src_t, i),
                                         in1=half(src_t, i))

                @step
                def _rows(i, cx):
                    nc.tensor.matmul(half(row_ps[:, 0, :], i),
                                     lhsT=onesr[:, 1:2], rhs=half(src_t, i),
                                     start=True, stop=True)
                    nc.tensor.matmul(half(row_ps[:, 1, :], i),
                                     lhsT=onesr[:, 0:1], rhs=half(s2, i),
                                     start=True, stop=True)

                @step
                def _mucp(i, cx):
                    # murow row 0: mean (for PE broadcast); Pool copy keeps
                    # DVE free for the variance math below
                    nc.gpsimd.tensor_copy(out=half(murow[:, 0, :], i),
                                          in_=half(row_ps[:, 0, :], i))

                @step
                def _var(i, cx):
                    nc.vector.tensor_scalar_mul(out=half(vr, i),
                                                in0=half(row_ps[:, 1, :], i),
                                                scalar1=1.0 / 128.0)

                @step
                def _m2(i, cx):
                    m2 = sml.tile([1, HQ], F32, name=nm + f"_m2{i}",
                                  tag=nm + f"m2{i}")
                    nc.vector.tensor_mul(out=m2, in0=half(row_ps[:, 0, :], i),
                                         in1=half(row_ps[:, 0, :], i))
                    nc.vector.tensor_sub(out=half(vr, i), in0=half(vr, i),
                                         in1=m2)

                @step
                def _sqrt(i, cx):
                    nc.scalar.activation(out=half(sd, i), in_=half(vr, i),
                                         func=SQRT, bias=eps_t[0:1, :],
                                         scale=1.0)

                @step
                def _rs(i, cx):
                    with nc.allow_low_precision(reason="f32r rstd"):
                        nc.vector.reciprocal(out=half(murow[:, 1, :], i),
                                             in_=half(sd, i))

                @step
                def _bcast(i, cx):
                    nc.tensor.matmul(half(mr_ps[:, 0, :], i), lhsT=ones_row,
                                     rhs=half(murow[:, 0, :], i),
                                     start=True, stop=True)
                    nc.tensor.matmul(half(mr_ps[:, 1, :], i), lhsT=ones_row,
                                     rhs=half(murow[:, 1, :], i),
                                     start=True, stop=True)

                @step
                def _zc(i, cx):
                    nc.vector.tensor_sub(out=half(zc, i), in0=half(src_t, i),
                                         in1=half(mr_ps[:, 0, :], i))

                @step
                def _zm(i, cx):
                    nc.vector.tensor_mul(out=half(zc, i), in0=half(zc, i),
                                         in1=half(mr_ps[:, 1, :], i))

                @step
                def _gb(i, cx):
                    nc.vector.tensor_scalar(out=half(dst_t, i),
                                            in0=half(zc, i), scalar1=gain,
                                            scalar2=bias_, op0=MULT, op1=ADD)

            zhat = finp.tile([D, QS], BF16, name="zhat")
            feat_ln_steps(zt, preg_t, preb_t, zhat, "ln1")

            gel = finp.tile([D, 2, QS], BF16, name="gel")
            res = finp.tile([D, QS], F32R, name="res")

            @step
            def _h1(i, cx):
                for f in range(2):
                    nc.tensor.matmul(half(h_ps[:, f, :], i),
                                     lhsT=w1_t[:, f * D:(f + 1) * D],
                                     rhs=half(zhat, i), start=True, stop=True)

            @step
            def _gelu(i, cx):
                for f in range(2):
                    nc.scalar.activation(out=half(gel[:, f, :], i),
                                         in_=half(h_ps[:, f, :], i),
                                         func=GELU, bias=b1_t[:, f:f + 1],
                                         scale=1.0)

            @step
            def _o2(i, cx):
                o2 = cx["o2"] = fps.tile([D, QS], F32, name=f"o2_ps",
                                         tag="zo")
                for f in range(2):
                    nc.tensor.matmul(half(o2, i), lhsT=w2_t[:, f, :],
                                     rhs=half(gel[:, f, :], i),
                                     start=(f == 0), stop=(f == 1))

            @step
            def _res(i, cx):
                nc.vector.tensor_scalar_add(out=half(res, i),
                                            in0=half(cx["o2"], i),
                                            scalar1=b2_t)

            @step
            def _res2(i, cx):
                nc.vector.tensor_add(out=half(res, i), in0=half(res, i),
                                     in1=half(zhat, i))

            final = finp.tile([D, QS], F32, name="final")
            feat_ln_steps(res, postg_t, postb_t, final, "ln2")

            @step
            def _out(i, cx):
                nc.sync.dma_start(out=half(out, i), in_=half(final, i))

            # interleaved emission: half 1 trails half 0 by one step
            ctxs = [{}, {}]
            o2_shared = [None]

            def run_steps():
                prev = None
                for f in steps:
                    f(0, ctxs[0])
                    if prev is not None:
                        prev(1, ctxs[1])
                    prev = f
                prev(1, ctxs[1])

            # o2 tile is tag-ring shared; allocate per half inside the step
            run_steps()
            fpool.__exit__(None, None, None)
            accpool.__exit__(None, None, None)

    if split:
        _split_sync_waits(nc)
    return nc


# ---------------------------------------------------------------------------
def _prep_core_inputs(b, r, q, k, v, skip, q_ln_g, q_ln_b, Wq, bq, k_ln_g,
                      k_ln_b, Wk, bk, v_ln_g, v_ln_b, Wv, bv, Wp, bp,
                      pre_g, pre_b, W1, b1, W2, b2, post_g, post_b):
    f32 = np.float32
    bf16 = ml_dtypes.bfloat16

    def fold(W, g):
        wg = g[:, None] * W
        return (wg - wg.sum(0, keepdims=True) / 128.0).astype(f32)

    wq_all = SCALE * fold(Wq, q_ln_g)                      # [D, 128]
    wk_all = fold(Wk, k_ln_g)                              # [D, 128]
    wv_f = fold(Wv, v_ln_g)                                # [D, 128]
    wv_ext = np.zeros((D, HEADS, 33), f32)
    wv_ext[:, :, 0:32] = wv_f.reshape(D, HEADS, DH)
    wcat = np.concatenate([
        wq_all, wk_all, wv_ext.reshape(D, HEADS * 33),
        W1.astype(f32),
        W2.reshape(2, D, D).transpose(1, 0, 2).reshape(D, 2 * D),
    ], axis=1).astype(bf16)

    # bias folding: q_ln_b -> wbq (added to q-heads); v_ln_b -> bp'
    # (rides through attention as a constant, then Wp); k_ln_b drops out
    # (adds a per-query constant to all logits -> softmax invariant).
    wbq = (SCALE * (Wq.T @ q_ln_b)).astype(f32)            # [128]
    wbv = Wv.T @ v_ln_b                                    # [128]
    bpp = bp + Wp.T @ wbv                                  # [D]
    pad = np.zeros(64, f32)
    fcon = np.stack([
        np.concatenate([wbq[0:64], pad]),
        np.concatenate([wbq[64:128], pad]),
        bpp, b1[0:D], b1[D:2 * D], b2,
        pre_g, pre_b, post_g, post_b,
    ], axis=1).astype(f32)

    sl = slice(r * QS, (r + 1) * QS)
    return {
        "xq": np.ascontiguousarray(
            q[b].reshape(NCAM, D, Q)[:, :, sl]).astype(bf16),
        "xk": np.ascontiguousarray(k[b].reshape(NCAM, D, KC)).astype(bf16),
        "xv": np.ascontiguousarray(v[b].reshape(NCAM, D, KC)).astype(bf16),
        "wcat": wcat,
        "wp": np.ascontiguousarray(
            Wp.reshape(HEADS, DH, D).transpose(1, 0, 2), f32),
        "fcon": fcon,
        "skipb": np.ascontiguousarray(skip[b].reshape(D, Q)[:, sl], f32),
    }


def kernel(**inputs):
    if "nc" not in _cached:
        _cached["nc"] = _build_program()
    nc = _cached["nc"]
    args = {kk: np.asarray(vv) for kk, vv in inputs.items()}
    in_maps = [_prep_core_inputs(c // 4, c % 4, **args) for c in range(N_CORES)]
    res = run_bass_kernel_spmd(nc, in_maps, core_ids=list(range(N_CORES)))
    full = np.zeros((B, D, Q), np.float32)
    for c in range(N_CORES):
        b, r = c // 4, c % 4
        full[b][:, r * QS:(r + 1) * QS] = res.results[c]["out"]
    return full.reshape(B, D, 32, 32)


# revision 15
# speedup vs baseline: 1.0421x; 1.0421x over previous
"""BEV cross-attention kernel for Trainium2, 8-core SPMD.

Shard: core c handles (batch b=c//4, query slice r=c%4 of 256 BEV queries),
computing ALL 4 heads for its queries. Keys/values (6 cams x 1680) are
replicated per core. No collectives: each core's output is a disjoint
[D, 256] token slice; the host concatenates.

Layout: feature-major ("S^T") attention - scores [keys=120p, (head, q)=1024f]
so softmax exp runs on ScalarE with per-partition (per-key) scale=rstd_k and
bias=ln(rstd_v) (K/V LayerNorms folded through the exp; shared by all heads).
LN means fold into centered projection weights host-side; the softmax
denominator rides the PV matmul as a per-head ones column of V. No max
subtraction (logits are small by construction).

Engine budget: ScalarE does the 84 exps (the wall, ~88us); PE does all
projections + QK/PV in bf16 (1 cyc/col); DVE does squares (bf16 2x) and
evacs; Pool does V evacs, cross-partition reduces and broadcasts. Per-token
LN stats are produced token-major directly by 1-col PE matmuls against a
ones vector (no DRAM bounces anywhere).
"""
import numpy as np
import ml_dtypes

import concourse.bass as bass
import concourse.bass_isa as bass_isa
import concourse.mybir as mybir
import concourse.tile as tile
from concourse.bass_utils import run_bass_kernel_spmd

F32 = mybir.dt.float32
F32R = mybir.dt.float32r
BF16 = mybir.dt.bfloat16

HEADS, DH, D = 4, 32, 128
B, NCAM = 2, 6
Q = 32 * 32            # 1024 BEV queries per batch
QS = Q // 4            # 256 queries per core
KC = 28 * 60           # 1680 keys per camera
CW = 120               # key chunk width: 1680 = 14 * 120, no tail
NKCH = KC // CW        # 14
N_CORES = 8
EPS = 1e-5
SCALE = DH ** -0.5

_cached = {}


# ---------------------------------------------------------------------------
# walrus compat: this container's walrus rejects instructions carrying more
# than one semaphore wait; move excess waits onto same-engine NoOps.
_COMPUTE_ENGINES = None
_nopctr = [0]


def _split_sync_waits(nc, limit=1):
    global _COMPUTE_ENGINES
    if _COMPUTE_ENGINES is None:
        _COMPUTE_ENGINES = {
            mybir.EngineType.PE, mybir.EngineType.Activation,
            mybir.EngineType.Pool, mybir.EngineType.DVE, mybir.EngineType.SP,
        }
    for f in nc.m.functions:
        for bb in f.blocks:
            out, changed = [], False
            for inst in bb.instructions:
                si = inst.sync_info
                if (si is not None and len(si.on_wait) > limit
                        and inst.engine in _COMPUTE_ENGINES):
                    waits = list(si.on_wait)
                    n_extra = len(waits) - limit
                    for i in range(0, n_extra, limit):
                        nop = mybir.InstNoOp(name=f"wait-split-{_nopctr[0]}")
                        _nopctr[0] += 1
                        nop.engine = inst.engine
                        nop.sync_info = mybir.SyncInfo(
                            on_wait=waits[i:min(i + limit, n_extra)], on_update=[])
                        out.append(nop)
                    si.on_wait = waits[n_extra:]
                    changed = True
                out.append(inst)
            if changed:
                bb.instructions = out


# ---------------------------------------------------------------------------
def _build_program(split=True, collective=True, n_dev=N_CORES):
    nc = bass.Bass("TRN2", target_bir_lowering=False, debug=False,
                   num_devices=n_dev)

    xq = nc.dram_tensor("xq", [NCAM, D, QS], BF16, kind="ExternalInput").ap()
    xk = nc.dram_tensor("xk", [NCAM, D, KC], BF16, kind="ExternalInput").ap()
    xv = nc.dram_tensor("xv", [NCAM, D, KC], BF16, kind="ExternalInput").ap()
    # packed bf16 weights: [wq 128 | wk 128 | wv_ext 132 | w1 256 | w2 256]
    wcat = nc.dram_tensor("wcat", [D, 900], BF16, kind="ExternalInput").ap()
    wp = nc.dram_tensor("wp", [DH, HEADS, D], F32R, kind="ExternalInput").ap()
    # packed f32 per-feature consts:
    # [wbq_pairA, wbq_pairB, bp', b1_0, b1_1, b2, pre_g, pre_b, post_g,
    #  post_b] (wbq pair columns hold heads 0-1 / 2-3 in partitions 0..63)
    fcon = nc.dram_tensor("fcon", [D, 10], F32, kind="ExternalInput").ap()
    skipb = nc.dram_tensor("skipb", [D, QS], F32, kind="ExternalInput").ap()

    out = nc.dram_tensor("out", [D, QS], F32, kind="ExternalOutput").ap()

    EXP = mybir.ActivationFunctionType.Exp
    LN_ = mybir.ActivationFunctionType.Ln
    SQRT = mybir.ActivationFunctionType.Sqrt
    GELU = mybir.ActivationFunctionType.Gelu
    ADD = mybir.AluOpType.add
    MULT = mybir.AluOpType.mult

    with tile.TileContext(nc) as tc:
        with tc.tile_pool(name="consts", bufs=1) as consts, \
             tc.tile_pool(name="loads", bufs=3) as loads, \
             tc.tile_pool(name="sq", bufs=2) as sqp, \
             tc.tile_pool(name="kv", bufs=3) as kvp, \
             tc.tile_pool(name="sml", bufs=2) as sml, \
             tc.tile_pool(name="ee", bufs=3) as eep, \
             tc.tile_pool(name="fin", bufs=1) as finp:

            # ---- constant tiles (DMAs are issued after the first
            # camera loads, in the schedule section) ----
            wcat_t = consts.tile([D, 900], BF16, name="wcat_t")
            wq_t = wcat_t[:, 0:128]
            wk_t = wcat_t[:, 128:256]
            wv_t = wcat_t[:, 256:388]          # [D, 4*33]
            w1_t = wcat_t[:, 388:644]
            w2_t = wcat_t[:, 644:900].rearrange("p (f d) -> p f d", f=2)
            wp_t = consts.tile([DH, HEADS, D], F32R, name="wp_t")
            fcon_t = consts.tile([D, 10], F32, name="fcon_t")
            wbq_ab = fcon_t[:, 0:2]
            bpp_t = fcon_t[:, 2:3]
            b1_t = fcon_t[:, 3:5]
            b2_t = fcon_t[:, 5:6]
            preg_t = fcon_t[:, 6:7]
            preb_t = fcon_t[:, 7:8]
            postg_t = fcon_t[:, 8:9]
            postb_t = fcon_t[:, 9:10]
            skip_t = consts.tile([D, QS], F32, name="skip_t")

            eps_t = consts.tile([D, 1], F32, name="eps_t")
            nc.vector.memset(eps_t, EPS)
            onesb = consts.tile([D, 2], BF16, name="onesb")  # [1 | 1/128]
            nc.vector.memset(onesb[:, 0:1], 1.0)
            nc.vector.memset(onesb[:, 1:2], 1.0 / 128.0)
            onesr = consts.tile([D, 2], F32R, name="onesr")  # [1 | 1/128]
            nc.vector.memset(onesr[:, 0:1], 1.0)
            nc.vector.memset(onesr[:, 1:2], 1.0 / 128.0)
            ones_row = consts.tile([1, D], F32R, name="ones_row")
            nc.vector.memset(ones_row, 1.0)

            # ---- PSUM pools ----
            # banks: avt 2 + sc 2x2 + kp 1 + shared proj 1 = 8
            accpool = tc.tile_pool(name="accp", bufs=1, space="PSUM")
            accp = accpool.__enter__()
            scpool = tc.tile_pool(name="scp", bufs=2, space="PSUM")
            scp = scpool.__enter__()
            projpool = tc.tile_pool(name="projp", bufs=1, space="PSUM")
            projp = projpool.__enter__()

            avt = accp.tile([33, HEADS, QS], F32, name="avt")      # 2 banks
            # bank A: kproj ping-pong [0:210|210:420] | stats [420:476]
            ka_ps = projp.tile([D, 512], F32, name="ka_ps")        # 1 bank
            kp_ps = [ka_ps[0:64, 0:210], ka_ps[0:64, 210:420]]
            st_ps = ka_ps[0:CW, 420:476].rearrange("p (j c) -> p j c", j=4)
            # bank B: vproj ping-pong [0:132|132:264] | qproj [256:512]
            sh_ps = projp.tile([D, 512], F32, name="sh_ps")        # 1 bank
            vp_ps = [sh_ps[0:CW, 0:132], sh_ps[0:CW, 132:264]]
            qp_ps = [sh_ps[0:64, 0:256], sh_ps[0:64, 256:512]]

            # ---- per-camera phase 1, split so the ScalarE ops (finish)
            # can be emitted mid-attention of the previous camera and never
            # block the exp stream ----
            def load(n):
                xk_t = loads.tile([D, KC], BF16, name="xk_t", tag="xk")
                nc.sync.dma_start(out=xk_t, in_=xk[n])
                xv_t = loads.tile([D, KC], BF16, name="xv_t", tag="xv")
                nc.sync.dma_start(out=xv_t, in_=xv[n])
                xq_t = loads.tile([D, QS], BF16, name="xq_t", tag="xq")
                nc.sync.dma_start(out=xq_t, in_=xq[n])
                return xk_t, xv_t, xq_t

            def produce(n, ld):
                xk_t, xv_t, xq_t = ld

                x2k = sqp.tile([D, KC], BF16, name="x2k", tag="x2k")
                nc.vector.tensor_mul(out=x2k, in0=xk_t, in1=xk_t)
                x2v = sqp.tile([D, KC], BF16, name="x2v", tag="x2v")
                nc.vector.tensor_mul(out=x2v, in0=xv_t, in1=xv_t)
                x2q = sqp.tile([D, QS], BF16, name="x2q", tag="x2q")
                nc.vector.tensor_mul(out=x2q, in0=xq_t, in1=xq_t)

                # token-major stats via 1-col matmuls:
                # st rows: 0=k-mean, 1=v-mean, 2=k-sumsq, 3=v-sumsq
                for c in range(NKCH):
                    xkc = xk_t[:, c * CW:(c + 1) * CW]
                    xvc = xv_t[:, c * CW:(c + 1) * CW]
                    x2kc = x2k[:, c * CW:(c + 1) * CW]
                    x2vc = x2v[:, c * CW:(c + 1) * CW]
                    nc.tensor.matmul(st_ps[:, 0, c:c + 1], lhsT=xkc,
                                     rhs=onesb[:, 1:2], start=True, stop=True)
                    nc.tensor.matmul(st_ps[:, 1, c:c + 1], lhsT=xvc,
                                     rhs=onesb[:, 1:2], start=True, stop=True)
                    nc.tensor.matmul(st_ps[:, 2, c:c + 1], lhsT=x2kc,
                                     rhs=onesb[:, 0:1], start=True, stop=True)
                    nc.tensor.matmul(st_ps[:, 3, c:c + 1], lhsT=x2vc,
                                     rhs=onesb[:, 0:1], start=True, stop=True)

                st_sb = sml.tile([CW, 4, NKCH], F32, name="st_sb", tag="st")
                nc.vector.tensor_copy(out=st_sb, in_=st_ps)
                mu2 = sml.tile([CW, 2, NKCH], F32, name="mu2", tag="mu2")
                nc.vector.tensor_mul(out=mu2, in0=st_sb[:, 0:2, :],
                                     in1=st_sb[:, 0:2, :])
                var2 = sml.tile([CW, 2, NKCH], F32, name="var2", tag="var2")
                nc.vector.tensor_scalar_mul(out=var2, in0=st_sb[:, 2:4, :],
                                            scalar1=1.0 / 128.0)
                nc.vector.tensor_sub(out=var2, in0=var2, in1=mu2)

                # q stats rows (Pool C-reduce, off the PE/Act path)
                musum = sml.tile([1, QS], F32, name="musum", tag="musum")
                nc.gpsimd.tensor_reduce(out=musum, in_=xq_t,
                                        axis=mybir.AxisListType.C, op=ADD)
                sssum = sml.tile([1, QS], F32, name="sssum", tag="sssum")
                nc.gpsimd.tensor_reduce(out=sssum, in_=x2q,
                                        axis=mybir.AxisListType.C, op=ADD)
                muq = sml.tile([1, QS], F32, name="muq", tag="muq")
                nc.vector.tensor_scalar_mul(out=muq, in0=musum,
                                            scalar1=1.0 / 128.0)
                mu2q = sml.tile([1, QS], F32, name="mu2q", tag="mu2q")
                nc.vector.tensor_mul(out=mu2q, in0=muq, in1=muq)
                varq = sml.tile([1, QS], F32, name="varq", tag="varq")
                nc.vector.tensor_scalar_mul(out=varq, in0=sssum,
                                            scalar1=1.0 / 128.0)
                nc.vector.tensor_sub(out=varq, in0=varq, in1=mu2q)

                # K projection, feature-major, split in head pairs so
                # attention lhsT slices sit at base partition 0/32
                khT = [kvp.tile([64, KC], BF16, name=f"khT{p}",
                                tag=f"khT{p}") for p in range(2)]
                for i, (j, p) in enumerate(
                        (j, p) for j in range(8) for p in range(2)):
                    kp = kp_ps[i % 2]
                    nc.tensor.matmul(
                        kp, lhsT=wk_t[:, p * 64:(p + 1) * 64],
                        rhs=xk_t[:, j * 210:(j + 1) * 210],
                        start=True, stop=True)
                    nc.vector.tensor_copy(
                        out=khT[p][:, j * 210:(j + 1) * 210], in_=kp)

                # V projection, token-major [120, 4, 33] per chunk
                # (col 32 of each head block is 0 from wv_ext; memset to 1
                # afterwards: softmax denominator ride-along)
                vhE = kvp.tile([CW, NKCH, HEADS, 33], BF16, name="vhE",
                               tag="vhE")
                nc.gpsimd.memset(vhE[:, :, :, 32], 1.0)
                for c in range(NKCH):
                    xvc = xv_t[:, c * CW:(c + 1) * CW]
                    vp = vp_ps[c % 2]
                    nc.tensor.matmul(vp, lhsT=xvc, rhs=wv_t,
                                     start=True, stop=True)
                    nc.gpsimd.tensor_copy(
                        out=vhE[:, c, :, 0:32],
                        in_=vp.rearrange("p (h d) -> p h d", h=4)[:, :, 0:32])
                return xq_t, var2, varq, khT, vhE

            def finish(n, prod):
                xq_t, var2, varq, khT, vhE = prod
                # ln(var+eps) for both K and V in one ScalarE op
                lnb = sml.tile([CW, 2, NKCH], F32, name="lnb", tag="lnb")
                nc.scalar.activation(out=lnb, in_=var2, func=LN_,
                                     bias=eps_t[0:CW, :], scale=1.0)
                rstdk = kvp.tile([CW, NKCH], F32, name="rstdk", tag="rstdk")
                nc.scalar.activation(out=rstdk, in_=lnb[:, 0, :], func=EXP,
                                     bias=0.0, scale=-0.5)
                lnrv = kvp.tile([CW, NKCH], F32, name="lnrv", tag="lnrv")
                nc.vector.tensor_scalar_mul(out=lnrv, in0=lnb[:, 1, :],
                                            scalar1=-0.5)

                sdq = sml.tile([1, QS], F32, name="sdq", tag="sdq")
                nc.scalar.activation(out=sdq, in_=varq, func=SQRT,
                                     bias=eps_t[0:1, :], scale=1.0)
                rqrow = sml.tile([1, QS], F32R, name="rqrow", tag="rqrow")
                with nc.allow_low_precision(reason="f32r rstd_q"):
                    nc.vector.reciprocal(out=rqrow, in_=sdq)
                rqbc = sh_ps[:, 0:256]
                nc.tensor.matmul(rqbc, lhsT=ones_row, rhs=rqrow,
                                 start=True, stop=True)
                # pre-scale x by rstd_q (commutes with the centered
                # projection), so the q matmuls have no ScalarE dependency
                xqn = sml.tile([D, QS], BF16, name="xqn", tag="xqn")
                nc.vector.tensor_mul(out=xqn, in0=xq_t, in1=rqbc)
                qhT = [kvp.tile([64, QS], BF16, name=f"qhT{p}",
                                tag=f"qhT{p}") for p in range(2)]
                for p in range(2):
                    nc.tensor.matmul(qp_ps[p],
                                     lhsT=wq_t[:, p * 64:(p + 1) * 64],
                                     rhs=xqn, start=True, stop=True)
                    nc.vector.tensor_scalar_add(
                        out=qhT[p], in0=qp_ps[p],
                        scalar1=wbq_ab[0:64, p:p + 1])
                return khT, vhE, rstdk, lnrv, qhT

            # ---- attention for one camera (chunk range) ----
            def attention(n, cam, c0, c1):
                khT, vhE, rstdk, lnrv, qhT = cam
                for c in range(c0, c1):
                    sc_ps = scp.tile([CW, HEADS, QS], F32, name="sc_ps",
                                     tag="sc")
                    for h in range(HEADS):
                        p, hh = divmod(h, 2)
                        nc.tensor.matmul(
                            sc_ps[:, h, :],
                            lhsT=khT[p][hh * DH:(hh + 1) * DH,
                                        c * CW:(c + 1) * CW],
                            rhs=qhT[p][hh * DH:(hh + 1) * DH, :],
                            start=True, stop=True)
                    et = eep.tile([CW, HEADS, QS], BF16, name="et", tag="et")
                    nc.scalar.activation(out=et, in_=sc_ps, func=EXP,
                                         bias=lnrv[:, c:c + 1],
                                         scale=rstdk[:, c:c + 1])
                    first = (n == 0 and c == 0)
                    last = (n == NCAM - 1 and c == NKCH - 1)
                    for h in range(HEADS):
                        nc.tensor.matmul(
                            avt[:, h, :],
                            lhsT=vhE[:, c, h, 0:33],
                            rhs=et[:, h, :],
                            start=first, stop=last)

            # ---- pipelined schedule: produce(n+1) and finish(n+1) are
            # emitted around the first half of attention(n) so no engine's
            # in-order queue ever blocks the exp stream ----
            cams = [None] * NCAM
            prods = [None] * NCAM
            ld0 = load(0)
            nc.sync.dma_start(out=wcat_t, in_=wcat)
            nc.sync.dma_start(out=fcon_t, in_=fcon)
            ld1 = load(1)
            nc.sync.dma_start(out=wp_t, in_=wp)
            nc.sync.dma_start(out=skip_t, in_=skipb)
            prods[0] = produce(0, ld0)
            cams[0] = finish(0, prods[0])
            attention(0, cams[0], 0, 3)
            prods[1] = produce(1, ld1)
            for n in range(NCAM):
                attention(n, cams[n], 3 if n == 0 else 0, 4)
                if n + 1 < NCAM:
                    cams[n + 1] = finish(n + 1, prods[n + 1])
                attention(n, cams[n], 4, 9)
                if n + 2 < NCAM:
                    prods[n + 2] = produce(n + 2, load(n + 2))
                attention(n, cams[n], 9, NKCH)

            # ---- tail: normalize, project, skip, LN, MLP, LN ----
            rden = finp.tile([1, HEADS, QS], F32R, name="rden")
            with nc.allow_low_precision(reason="f32r denominator"):
                nc.vector.reciprocal(out=rden, in_=avt[32:33, :, :])

            projpool.__exit__(None, None, None)
            scpool.__exit__(None, None, None)
            fpool = tc.tile_pool(name="fps", bufs=1, space="PSUM")
            fps = fpool.__enter__()
            rd_ps = fps.tile([DH, 512], F32, name="rd_ps")
            anorm = finp.tile([DH, HEADS, QS], F32R, name="anorm")
            rden_f = rden.rearrange("p h q -> p (h q)")
            anorm_f = anorm.rearrange("p h q -> p (h q)")
            avt_f = avt[0:32, :, :].rearrange("p h q -> p (h q)")
            for j in range(2):
                nc.tensor.matmul(rd_ps, lhsT=ones_row[:, 0:DH],
                                 rhs=rden_f[:, j * 512:(j + 1) * 512],
                                 start=True, stop=True)
                nc.vector.tensor_mul(out=anorm_f[:, j * 512:(j + 1) * 512],
                                     in0=avt_f[:, j * 512:(j + 1) * 512],
                                     in1=rd_ps)

            zo_ps = fps.tile([D, QS], F32, name="zo_ps", tag="zo")
            for h in range(HEADS):
                nc.tensor.matmul(zo_ps, lhsT=wp_t[:, h, :],
                                 rhs=anorm[:, h, :],
                                 start=(h == 0), stop=(h == HEADS - 1))
            zt = finp.tile([D, QS], F32R, name="zt")
            nc.vector.tensor_add(out=zt, in0=zo_ps, in1=skip_t)
            nc.vector.tensor_scalar_add(out=zt, in0=zt, scalar1=bpp_t)

            row_ps = fps.tile([1, 2, QS], F32, name="row_ps")
            mr_ps = fps.tile([D, 2, QS], F32, name="mr_ps")

            def feat_ln(src, gain, bias_, dst_dt, nm):
                """LayerNorm across partitions (d) of src [128, QS].
                Critical path: s2 -> rows -> vr -> sub -> sqrt -> recip ->
                bcast -> zm -> gb; the mean copy/broadcast and zc run off
                the path on Pool/PE/DVE slack."""
                s2 = finp.tile([D, QS], F32R, name=nm + "_s2", tag="ln_s2")
                nc.vector.tensor_mul(out=s2, in0=src, in1=src)
                nc.tensor.matmul(row_ps[:, 0, :], lhsT=onesr[:, 1:2],
                                 rhs=src, start=True, stop=True)
                nc.tensor.matmul(row_ps[:, 1, :], lhsT=onesr[:, 0:1],
                                 rhs=s2, start=True, stop=True)
                murow = sml.tile([1, QS], F32R, name=nm + "_mu", tag="ln_mu")
                nc.gpsimd.tensor_copy(out=murow, in_=row_ps[:, 0, :])
                m2 = sml.tile([1, QS], F32, name=nm + "_m2", tag="ln_m2")
                nc.gpsimd.tensor_mul(out=m2, in0=row_ps[:, 0, :],
                                     in1=row_ps[:, 0, :])
                vr = sml.tile([1, QS], F32, name=nm + "_vr", tag="ln_vr")
                nc.vector.tensor_scalar_mul(out=vr, in0=row_ps[:, 1, :],
                                            scalar1=1.0 / 128.0)
                nc.vector.tensor_sub(out=vr, in0=vr, in1=m2)
                sd = sml.tile([1, QS], F32, name=nm + "_sd", tag="ln_sd")
                nc.scalar.activation(out=sd, in_=vr, func=SQRT,
                                     bias=eps_t[0:1, :], scale=1.0)
                rs = sml.tile([1, QS], F32R, name=nm + "_rs", tag="ln_rs")
                with nc.allow_low_precision(reason="f32r rstd"):
                    nc.vector.reciprocal(out=rs, in_=sd)
                nc.tensor.matmul(mr_ps[:, 0, :], lhsT=ones_row,
                                 rhs=murow, start=True, stop=True)
                zc = finp.tile([D, QS], F32R, name=nm + "_zc", tag="ln_zc")
                nc.vector.tensor_sub(out=zc, in0=src, in1=mr_ps[:, 0, :])
                nc.tensor.matmul(mr_ps[:, 1, :], lhsT=ones_row,
                                 rhs=rs, start=True, stop=True)
                zm = finp.tile([D, QS], F32R, name=nm + "_zm", tag="ln_zm")
                nc.vector.tensor_mul(out=zm, in0=zc, in1=mr_ps[:, 1, :])
                dst = finp.tile([D, QS], dst_dt, name=nm, tag="ln_dst")
                nc.vector.tensor_scalar(out=dst, in0=zm, scalar1=gain,
                                        scalar2=bias_, op0=MULT, op1=ADD)
                return dst

            zhat = feat_ln(zt, preg_t, preb_t, BF16, "zhat")

            h_ps = fps.tile([D, 2, QS], F32, name="h_ps")
            for f in range(2):
                nc.tensor.matmul(h_ps[:, f, :],
                                 lhsT=w1_t[:, f * D:(f + 1) * D],
                                 rhs=zhat, start=True, stop=True)
            gel = finp.tile([D, 2, QS], BF16, name="gel")
            for f in range(2):
                nc.scalar.activation(out=gel[:, f, :], in_=h_ps[:, f, :],
                                     func=GELU, bias=b1_t[:, f:f + 1],
                                     scale=1.0)
            o2_ps = fps.tile([D, QS], F32, name="o2_ps", tag="zo")
            for f in range(2):
                nc.tensor.matmul(o2_ps, lhsT=w2_t[:, f, :], rhs=gel[:, f, :],
                                 start=(f == 0), stop=(f == 1))
            res = finp.tile([D, QS], F32R, name="res")
            nc.vector.tensor_scalar_add(out=res, in0=o2_ps, scalar1=b2_t)
            nc.vector.tensor_add(out=res, in0=res, in1=zhat)

            final = feat_ln(res, postg_t, postb_t, F32, "final")
            nc.sync.dma_start(out=out, in_=final)
            fpool.__exit__(None, None, None)
            accpool.__exit__(None, None, None)

    if split:
        _split_sync_waits(nc)
    return nc


# ---------------------------------------------------------------------------
def _prep_core_inputs(b, r, q, k, v, skip, q_ln_g, q_ln_b, Wq, bq, k_ln_g,
                      k_ln_b, Wk, bk, v_ln_g, v_ln_b, Wv, bv, Wp, bp,
                      pre_g, pre_b, W1, b1, W2, b2, post_g, post_b):
    f32 = np.float32
    bf16 = ml_dtypes.bfloat16

    def fold(W, g):
        wg = g[:, None] * W
        return (wg - wg.sum(0, keepdims=True) / 128.0).astype(f32)

    wq_all = SCALE * fold(Wq, q_ln_g)                      # [D, 128]
    wk_all = fold(Wk, k_ln_g)                              # [D, 128]
    wv_f = fold(Wv, v_ln_g)                                # [D, 128]
    wv_ext = np.zeros((D, HEADS, 33), f32)
    wv_ext[:, :, 0:32] = wv_f.reshape(D, HEADS, DH)
    wcat = np.concatenate([
        wq_all, wk_all, wv_ext.reshape(D, HEADS * 33),
        W1.astype(f32),
        W2.reshape(2, D, D).transpose(1, 0, 2).reshape(D, 2 * D),
    ], axis=1).astype(bf16)

    # bias folding: q_ln_b -> wbq (added to q-heads); v_ln_b -> bp'
    # (rides through attention as a constant, then Wp); k_ln_b drops out
    # (adds a per-query constant to all logits -> softmax invariant).
    wbq = (SCALE * (Wq.T @ q_ln_b)).astype(f32)            # [128]
    wbv = Wv.T @ v_ln_b                                    # [128]
    bpp = bp + Wp.T @ wbv                                  # [D]
    pad = np.zeros(64, f32)
    fcon = np.stack([
        np.concatenate([wbq[0:64], pad]),
        np.concatenate([wbq[64:128], pad]),
        bpp, b1[0:D], b1[D:2 * D], b2,
        pre_g, pre_b, post_g, post_b,
    ], axis=1).astype(f32)

    sl = slice(r * QS, (r + 1) * QS)
    return {
        "xq": np.ascontiguousarray(
            q[b].reshape(NCAM, D, Q)[:, :, sl]).astype(bf16),
        "xk": np.ascontiguousarray(k[b].reshape(NCAM, D, KC)).astype(bf16),
        "xv": np.ascontiguousarray(v[b].reshape(NCAM, D, KC)).astype(bf16),
        "wcat": wcat,
        "wp": np.ascontiguousarray(
            Wp.reshape(HEADS, DH, D).transpose(1, 0, 2), f32),
        "fcon": fcon,
        "skipb": np.ascontiguousarray(skip[b].reshape(D, Q)[:, sl], f32),
    }


def kernel(**inputs):
    if "nc" not in _cached:
        _cached["nc"] = _build_program()
    nc = _cached["nc"]
    args = {kk: np.asarray(vv) for kk, vv in inputs.items()}
    in_maps = [_prep_core_inputs(c // 4, c % 4, **args) for c in range(N_CORES)]
    res = run_bass_kernel_spmd(nc, in_maps, core_ids=list(range(N_CORES)))
    full = np.zeros((B, D, Q), np.float32)
    for c in range(N_CORES):
        b, r = c // 4, c % 4
        full[b][:, r * QS:(r + 1) * QS] = res.results[c]["out"]
    return full.reshape(B, D, 32, 32)


# revision 18
# speedup vs baseline: 1.0719x; 1.0287x over previous
"""BEV cross-attention kernel for Trainium2, 8-core SPMD.

Shard: core c handles (batch b=c//4, query slice r=c%4 of 256 BEV queries),
computing ALL 4 heads for its queries. Keys/values (6 cams x 1680) are
replicated per core. No collectives: each core's output is a disjoint
[D, 256] token slice; the host concatenates.

Layout: feature-major ("S^T") attention - scores [keys=120p, (head, q)=1024f]
so softmax exp runs on ScalarE with per-partition (per-key) scale=rstd_k and
bias=ln(rstd_v) (K/V LayerNorms folded through the exp; shared by all heads).
LN means fold into centered projection weights host-side; the softmax
denominator rides the PV matmul as a per-head ones column of V. No max
subtraction (logits are small by construction).

Engine budget: ScalarE does the 84 exps (the wall, ~88us); PE does all
projections + QK/PV in bf16 (1 cyc/col); DVE does squares (bf16 2x) and
evacs; Pool does V evacs, cross-partition reduces and broadcasts. Per-token
LN stats are produced token-major directly by 1-col PE matmuls against a
ones vector (no DRAM bounces anywhere).
"""
import numpy as np
import ml_dtypes

import concourse.bass as bass
import concourse.bass_isa as bass_isa
import concourse.mybir as mybir
import concourse.tile as tile
from concourse.bass_utils import run_bass_kernel_spmd

F32 = mybir.dt.float32
F32R = mybir.dt.float32r
BF16 = mybir.dt.bfloat16

HEADS, DH, D = 4, 32, 128
B, NCAM = 2, 6
Q = 32 * 32            # 1024 BEV queries per batch
QS = Q // 4            # 256 queries per core
KC = 28 * 60           # 1680 keys per camera
CW = 120               # key chunk width: 1680 = 14 * 120, no tail
NKCH = KC // CW        # 14
N_CORES = 8
EPS = 1e-5
SCALE = DH ** -0.5

_cached = {}


# ---------------------------------------------------------------------------
# walrus compat: this container's walrus rejects instructions carrying more
# than one semaphore wait; move excess waits onto same-engine NoOps.
_COMPUTE_ENGINES = None
_nopctr = [0]


def _split_sync_waits(nc, limit=1):
    global _COMPUTE_ENGINES
    if _COMPUTE_ENGINES is None:
        _COMPUTE_ENGINES = {
            mybir.EngineType.PE, mybir.EngineType.Activation,
            mybir.EngineType.Pool, mybir.EngineType.DVE, mybir.EngineType.SP,
        }
    for f in nc.m.functions:
        for bb in f.blocks:
            out, changed = [], False
            for inst in bb.instructions:
                si = inst.sync_info
                if (si is not None and len(si.on_wait) > limit
                        and inst.engine in _COMPUTE_ENGINES):
                    waits = list(si.on_wait)
                    n_extra = len(waits) - limit
                    for i in range(0, n_extra, limit):
                        nop = mybir.InstNoOp(name=f"wait-split-{_nopctr[0]}")
                        _nopctr[0] += 1
                        nop.engine = inst.engine
                        nop.sync_info = mybir.SyncInfo(
                            on_wait=waits[i:min(i + limit, n_extra)], on_update=[])
                        out.append(nop)
                    si.on_wait = waits[n_extra:]
                    changed = True
                out.append(inst)
            if changed:
                bb.instructions = out


# ---------------------------------------------------------------------------
def _build_program(split=True, collective=True, n_dev=N_CORES):
    nc = bass.Bass("TRN2", target_bir_lowering=False, debug=False,
                   num_devices=n_dev)

    xq = nc.dram_tensor("xq", [NCAM, D, QS], BF16, kind="ExternalInput").ap()
    xk = nc.dram_tensor("xk", [NCAM, D, KC], BF16, kind="ExternalInput").ap()
    xv = nc.dram_tensor("xv", [NCAM, D, KC], BF16, kind="ExternalInput").ap()
    # packed bf16 weights: [wq 128 | wk 128 | wv_ext 132 | w1 256 | w2 256]
    wcat = nc.dram_tensor("wcat", [D, 900], BF16, kind="ExternalInput").ap()
    wp = nc.dram_tensor("wp", [DH, HEADS, D], F32R, kind="ExternalInput").ap()
    # packed f32 per-feature consts:
    # [wbq_pairA, wbq_pairB, bp', b1_0, b1_1, b2, pre_g, pre_b, post_g,
    #  post_b] (wbq pair columns hold heads 0-1 / 2-3 in partitions 0..63)
    fcon = nc.dram_tensor("fcon", [D, 10], F32, kind="ExternalInput").ap()
    skipb = nc.dram_tensor("skipb", [D, QS], F32, kind="ExternalInput").ap()

    out = nc.dram_tensor("out", [D, QS], F32, kind="ExternalOutput").ap()

    EXP = mybir.ActivationFunctionType.Exp
    LN_ = mybir.ActivationFunctionType.Ln
    SQRT = mybir.ActivationFunctionType.Sqrt
    GELU = mybir.ActivationFunctionType.Gelu
    ADD = mybir.AluOpType.add
    MULT = mybir.AluOpType.mult

    with tile.TileContext(nc) as tc:
        with tc.tile_pool(name="consts", bufs=1) as consts, \
             tc.tile_pool(name="loads", bufs=3) as loads, \
             tc.tile_pool(name="sq", bufs=2) as sqp, \
             tc.tile_pool(name="kv", bufs=3) as kvp, \
             tc.tile_pool(name="sml", bufs=2) as sml, \
             tc.tile_pool(name="ee", bufs=3) as eep, \
             tc.tile_pool(name="fin", bufs=1) as finp:

            # ---- constant tiles (DMAs are issued after the first
            # camera loads, in the schedule section) ----
            wcat_t = consts.tile([D, 900], BF16, name="wcat_t")
            wq_t = wcat_t[:, 0:128]
            wk_t = wcat_t[:, 128:256]
            wv_t = wcat_t[:, 256:388]          # [D, 4*33]
            w1_t = wcat_t[:, 388:644]
            w2_t = wcat_t[:, 644:900].rearrange("p (f d) -> p f d", f=2)
            wp_t = consts.tile([DH, HEADS, D], F32R, name="wp_t")
            fcon_t = consts.tile([D, 10], F32, name="fcon_t")
            wbq_ab = fcon_t[:, 0:2]
            bpp_t = fcon_t[:, 2:3]
            b1_t = fcon_t[:, 3:5]
            b2_t = fcon_t[:, 5:6]
            preg_t = fcon_t[:, 6:7]
            preb_t = fcon_t[:, 7:8]
            postg_t = fcon_t[:, 8:9]
            postb_t = fcon_t[:, 9:10]
            skip_t = consts.tile([D, QS], F32, name="skip_t")

            eps_t = consts.tile([D, 1], F32, name="eps_t")
            nc.vector.memset(eps_t, EPS)
            onesb = consts.tile([D, 2], BF16, name="onesb")  # [1 | 1/128]
            nc.vector.memset(onesb[:, 0:1], 1.0)
            nc.vector.memset(onesb[:, 1:2], 1.0 / 128.0)
            onesr = consts.tile([D, 2], F32R, name="onesr")  # [1 | 1/128]
            nc.vector.memset(onesr[:, 0:1], 1.0)
            nc.vector.memset(onesr[:, 1:2], 1.0 / 128.0)
            ones_row = consts.tile([1, D], F32R, name="ones_row")
            nc.vector.memset(ones_row, 1.0)

            # ---- PSUM pools ----
            # banks: avt 2 + sc 2x2 + kp 1 + shared proj 1 = 8
            accpool = tc.tile_pool(name="accp", bufs=1, space="PSUM")
            accp = accpool.__enter__()
            scpool = tc.tile_pool(name="scp", bufs=2, space="PSUM")
            scp = scpool.__enter__()
            projpool = tc.tile_pool(name="projp", bufs=1, space="PSUM")
            projp = projpool.__enter__()

            avt = accp.tile([33, HEADS, QS], F32, name="avt")      # 2 banks
            # bank A: kproj ping-pong [0:210|210:420] | stats [420:476]
            ka_ps = projp.tile([D, 512], F32, name="ka_ps")        # 1 bank
            kp_ps = [ka_ps[0:64, 0:210], ka_ps[0:64, 210:420]]
            st_ps = ka_ps[0:CW, 420:476].rearrange("p (j c) -> p j c", j=4)
            # bank B: vproj ping-pong [0:132|132:264] | qproj [256:512]
            sh_ps = projp.tile([D, 512], F32, name="sh_ps")        # 1 bank
            vp_ps = [sh_ps[0:CW, 0:132], sh_ps[0:CW, 132:264]]
            qp_ps = [sh_ps[0:64, 0:256], sh_ps[0:64, 256:512]]

            # ---- per-camera phase 1, split so the ScalarE ops (finish)
            # can be emitted mid-attention of the previous camera and never
            # block the exp stream ----
            def load(n):
                xk_t = loads.tile([D, KC], BF16, name="xk_t", tag="xk")
                nc.sync.dma_start(out=xk_t, in_=xk[n])
                xv_t = loads.tile([D, KC], BF16, name="xv_t", tag="xv")
                nc.sync.dma_start(out=xv_t, in_=xv[n])
                xq_t = loads.tile([D, QS], BF16, name="xq_t", tag="xq")
                nc.sync.dma_start(out=xq_t, in_=xq[n])
                return xk_t, xv_t, xq_t

            def produce_a(n, ld):
                xk_t, xv_t, xq_t = ld

                x2k = sqp.tile([D, KC], BF16, name="x2k", tag="x2k")
                nc.vector.tensor_mul(out=x2k, in0=xk_t, in1=xk_t)
                x2v = sqp.tile([D, KC], BF16, name="x2v", tag="x2v")
                nc.vector.tensor_mul(out=x2v, in0=xv_t, in1=xv_t)
                x2q = sqp.tile([D, QS], BF16, name="x2q", tag="x2q")
                nc.vector.tensor_mul(out=x2q, in0=xq_t, in1=xq_t)

                # token-major stats via 1-col matmuls:
                # st rows: 0=k-mean, 1=v-mean, 2=k-sumsq, 3=v-sumsq
                for c in range(NKCH):
                    xkc = xk_t[:, c * CW:(c + 1) * CW]
                    xvc = xv_t[:, c * CW:(c + 1) * CW]
                    x2kc = x2k[:, c * CW:(c + 1) * CW]
                    x2vc = x2v[:, c * CW:(c + 1) * CW]
                    nc.tensor.matmul(st_ps[:, 0, c:c + 1], lhsT=xkc,
                                     rhs=onesb[:, 1:2], start=True, stop=True)
                    nc.tensor.matmul(st_ps[:, 1, c:c + 1], lhsT=xvc,
                                     rhs=onesb[:, 1:2], start=True, stop=True)
                    nc.tensor.matmul(st_ps[:, 2, c:c + 1], lhsT=x2kc,
                                     rhs=onesb[:, 0:1], start=True, stop=True)
                    nc.tensor.matmul(st_ps[:, 3, c:c + 1], lhsT=x2vc,
                                     rhs=onesb[:, 0:1], start=True, stop=True)

                st_sb = sml.tile([CW, 4, NKCH], F32, name="st_sb", tag="st")
                nc.vector.tensor_copy(out=st_sb, in_=st_ps)
                mu2 = sml.tile([CW, 2, NKCH], F32, name="mu2", tag="mu2")
                nc.vector.tensor_mul(out=mu2, in0=st_sb[:, 0:2, :],
                                     in1=st_sb[:, 0:2, :])
                var2 = sml.tile([CW, 2, NKCH], F32, name="var2", tag="var2")
                nc.vector.tensor_scalar_mul(out=var2, in0=st_sb[:, 2:4, :],
                                            scalar1=1.0 / 128.0)
                nc.vector.tensor_sub(out=var2, in0=var2, in1=mu2)

                # q stats rows (Pool C-reduce, off the PE/Act path)
                musum = sml.tile([1, QS], F32, name="musum", tag="musum")
                nc.gpsimd.tensor_reduce(out=musum, in_=xq_t,
                                        axis=mybir.AxisListType.C, op=ADD)
                sssum = sml.tile([1, QS], F32, name="sssum", tag="sssum")
                nc.gpsimd.tensor_reduce(out=sssum, in_=x2q,
                                        axis=mybir.AxisListType.C, op=ADD)
                muq = sml.tile([1, QS], F32, name="muq", tag="muq")
                nc.vector.tensor_scalar_mul(out=muq, in0=musum,
                                            scalar1=1.0 / 128.0)
                mu2q = sml.tile([1, QS], F32, name="mu2q", tag="mu2q")
                nc.vector.tensor_mul(out=mu2q, in0=muq, in1=muq)
                varq = sml.tile([1, QS], F32, name="varq", tag="varq")
                nc.vector.tensor_scalar_mul(out=varq, in0=sssum,
                                            scalar1=1.0 / 128.0)
                nc.vector.tensor_sub(out=varq, in0=varq, in1=mu2q)

                return xk_t, xv_t, xq_t, var2, varq

            def produce_b(n, pa):
                xk_t, xv_t, xq_t, var2, varq = pa
                # K projection, feature-major, split in head pairs so
                # attention lhsT slices sit at base partition 0/32
                khT = [kvp.tile([64, KC], BF16, name=f"khT{p}",
                                tag=f"khT{p}") for p in range(2)]
                for i, (j, p) in enumerate(
                        (j, p) for j in range(8) for p in range(2)):
                    kp = kp_ps[i % 2]
                    nc.tensor.matmul(
                        kp, lhsT=wk_t[:, p * 64:(p + 1) * 64],
                        rhs=xk_t[:, j * 210:(j + 1) * 210],
                        start=True, stop=True)
                    nc.vector.tensor_copy(
                        out=khT[p][:, j * 210:(j + 1) * 210], in_=kp)

                # V projection, token-major [120, 4, 33] per chunk
                # (col 32 of each head block is 0 from wv_ext; memset to 1
                # afterwards: softmax denominator ride-along)
                vhE = kvp.tile([CW, NKCH, HEADS, 33], BF16, name="vhE",
                               tag="vhE")
                nc.gpsimd.memset(vhE[:, :, :, 32], 1.0)
                for c in range(NKCH):
                    xvc = xv_t[:, c * CW:(c + 1) * CW]
                    vp = vp_ps[c % 2]
                    nc.tensor.matmul(vp, lhsT=xvc, rhs=wv_t,
                                     start=True, stop=True)
                    nc.vector.tensor_copy(
                        out=vhE[:, c, :, 0:32],
                        in_=vp.rearrange("p (h d) -> p h d", h=4)[:, :, 0:32])
                return khT, vhE

            def produce(n, ld):
                pa = produce_a(n, ld)
                khT, vhE = produce_b(n, pa)
                return pa[2], pa[3], pa[4], khT, vhE

            def finish(n, prod):
                xq_t, var2, varq, khT, vhE = prod
                # ln(var+eps) for both K and V in one ScalarE op
                lnb = sml.tile([CW, 2, NKCH], F32, name="lnb", tag="lnb")
                nc.scalar.activation(out=lnb, in_=var2, func=LN_,
                                     bias=eps_t[0:CW, :], scale=1.0)
                rstdk = kvp.tile([CW, NKCH], F32, name="rstdk", tag="rstdk")
                nc.scalar.activation(out=rstdk, in_=lnb[:, 0, :], func=EXP,
                                     bias=0.0, scale=-0.5)
                lnrv = kvp.tile([CW, NKCH], F32, name="lnrv", tag="lnrv")
                nc.vector.tensor_scalar_mul(out=lnrv, in0=lnb[:, 1, :],
                                            scalar1=-0.5)

                sdq = sml.tile([1, QS], F32, name="sdq", tag="sdq")
                nc.scalar.activation(out=sdq, in_=varq, func=SQRT,
                                     bias=eps_t[0:1, :], scale=1.0)
                rqrow = sml.tile([1, QS], F32R, name="rqrow", tag="rqrow")
                with nc.allow_low_precision(reason="f32r rstd_q"):
                    nc.vector.reciprocal(out=rqrow, in_=sdq)
                rqbc = sh_ps[:, 0:256]
                nc.tensor.matmul(rqbc, lhsT=ones_row, rhs=rqrow,
                                 start=True, stop=True)
                # pre-scale x by rstd_q (commutes with the centered
                # projection), so the q matmuls have no ScalarE dependency
                xqn = sml.tile([D, QS], BF16, name="xqn", tag="xqn")
                nc.vector.tensor_mul(out=xqn, in0=xq_t, in1=rqbc)
                qhT = [kvp.tile([64, QS], BF16, name=f"qhT{p}",
                                tag=f"qhT{p}") for p in range(2)]
                for p in range(2):
                    nc.tensor.matmul(qp_ps[p],
                                     lhsT=wq_t[:, p * 64:(p + 1) * 64],
                                     rhs=xqn, start=True, stop=True)
                    nc.vector.tensor_scalar_add(
                        out=qhT[p], in0=qp_ps[p],
                        scalar1=wbq_ab[0:64, p:p + 1])
                return khT, vhE, rstdk, lnrv, qhT

            # ---- attention for one camera (chunk range) ----
            def attention(n, cam, c0, c1):
                khT, vhE, rstdk, lnrv, qhT = cam
                for c in range(c0, c1):
                    sc_ps = scp.tile([CW, HEADS, QS], F32, name="sc_ps",
                                     tag="sc")
                    for h in range(HEADS):
                        p, hh = divmod(h, 2)
                        nc.tensor.matmul(
                            sc_ps[:, h, :],
                            lhsT=khT[p][hh * DH:(hh + 1) * DH,
                                        c * CW:(c + 1) * CW],
                            rhs=qhT[p][hh * DH:(hh + 1) * DH, :],
                            start=True, stop=True)
                    et = eep.tile([CW, HEADS, QS], BF16, name="et", tag="et")
                    nc.scalar.activation(out=et, in_=sc_ps, func=EXP,
                                         bias=lnrv[:, c:c + 1],
                                         scale=rstdk[:, c:c + 1])
                    first = (n == 0 and c == 0)
                    last = (n == NCAM - 1 and c == NKCH - 1)
                    for h in range(HEADS):
                        nc.tensor.matmul(
                            avt[:, h, :],
                            lhsT=vhE[:, c, h, 0:33],
                            rhs=et[:, h, :],
                            start=first, stop=last)

            # ---- pipelined schedule: produce(n+1) and finish(n+1) are
            # emitted around the first half of attention(n) so no engine's
            # in-order queue ever blocks the exp stream ----
            cams = [None] * NCAM
            prods = [None] * NCAM
            ld0 = load(0)
            nc.sync.dma_start(out=wcat_t, in_=wcat)
            nc.sync.dma_start(out=fcon_t, in_=fcon)
            ld1 = load(1)
            nc.sync.dma_start(out=wp_t, in_=wp)
            nc.sync.dma_start(out=skip_t, in_=skipb)
            pa0 = produce_a(0, ld0)
            fin0 = finish(0, (pa0[2], pa0[3], pa0[4], None, None))
            khT0, vhE0 = produce_b(0, pa0)
            cams[0] = (khT0, vhE0, fin0[2], fin0[3], fin0[4])
            attention(0, cams[0], 0, 3)
            prods[1] = produce(1, ld1)
            for n in range(NCAM):
                attention(n, cams[n], 3 if n == 0 else 0, 4)
                if n + 1 < NCAM:
                    cams[n + 1] = finish(n + 1, prods[n + 1])
                attention(n, cams[n], 4, 9)
                if n + 2 < NCAM:
                    prods[n + 2] = produce(n + 2, load(n + 2))
                attention(n, cams[n], 9, NKCH)

            # ---- tail: normalize, project, skip, LN, MLP, LN ----
            rden = finp.tile([1, HEADS, QS], F32R, name="rden")
            with nc.allow_low_precision(reason="f32r denominator"):
                nc.vector.reciprocal(out=rden, in_=avt[32:33, :, :])

            projpool.__exit__(None, None, None)
            scpool.__exit__(None, None, None)
            fpool = tc.tile_pool(name="fps", bufs=1, space="PSUM")
            fps = fpool.__enter__()
            rd_ps = fps.tile([DH, 512], F32, name="rd_ps")
            avt_sb = finp.tile([DH, HEADS, QS], F32, name="avt_sb")
            nc.vector.tensor_copy(out=avt_sb, in_=avt[0:32, :, :])
            anorm = finp.tile([DH, HEADS, QS], F32R, name="anorm")
            rden_f = rden.rearrange("p h q -> p (h q)")
            anorm_f = anorm.rearrange("p h q -> p (h q)")
            avt_f = avt_sb.rearrange("p h q -> p (h q)")
            for j in range(2):
                nc.tensor.matmul(rd_ps, lhsT=ones_row[:, 0:DH],
                                 rhs=rden_f[:, j * 512:(j + 1) * 512],
                                 start=True, stop=True)
                nc.vector.tensor_mul(out=anorm_f[:, j * 512:(j + 1) * 512],
                                     in0=avt_f[:, j * 512:(j + 1) * 512],
                                     in1=rd_ps)

            zo_ps = fps.tile([D, QS], F32, name="zo_ps", tag="zo")
            for h in range(HEADS):
                nc.tensor.matmul(zo_ps, lhsT=wp_t[:, h, :],
                                 rhs=anorm[:, h, :],
                                 start=(h == 0), stop=(h == HEADS - 1))
            zt = finp.tile([D, QS], F32R, name="zt")
            nc.vector.tensor_add(out=zt, in0=zo_ps, in1=skip_t)
            nc.vector.tensor_scalar_add(out=zt, in0=zt, scalar1=bpp_t)

            row_ps = fps.tile([1, 2, QS], F32, name="row_ps")
            mr_ps = fps.tile([D, 2, QS], F32, name="mr_ps")

            def feat_ln(src, gain, bias_, dst_dt, nm):
                """LayerNorm across partitions (d) of src [128, QS].
                Critical path: s2 -> rows -> vr -> sub -> sqrt -> recip ->
                bcast -> zm -> gb; the mean copy/broadcast and zc run off
                the path on Pool/PE/DVE slack."""
                s2 = finp.tile([D, QS], F32R, name=nm + "_s2", tag="ln_s2")
                nc.vector.tensor_mul(out=s2, in0=src, in1=src)
                nc.tensor.matmul(row_ps[:, 0, :], lhsT=onesr[:, 1:2],
                                 rhs=src, start=True, stop=True)
                nc.tensor.matmul(row_ps[:, 1, :], lhsT=onesr[:, 0:1],
                                 rhs=s2, start=True, stop=True)
                murow = sml.tile([1, QS], F32R, name=nm + "_mu", tag="ln_mu")
                nc.scalar.copy(out=murow, in_=row_ps[:, 0, :])
                m2 = sml.tile([1, QS], F32, name=nm + "_m2", tag="ln_m2")
                nc.scalar.square(out=m2, in_=row_ps[:, 0, :])
                vr = sml.tile([1, QS], F32, name=nm + "_vr", tag="ln_vr")
                nc.vector.tensor_scalar_mul(out=vr, in0=row_ps[:, 1, :],
                                            scalar1=1.0 / 128.0)
                nc.vector.tensor_sub(out=vr, in0=vr, in1=m2)
                sd = sml.tile([1, QS], F32, name=nm + "_sd", tag="ln_sd")
                nc.scalar.activation(out=sd, in_=vr, func=SQRT,
                                     bias=eps_t[0:1, :], scale=1.0)
                rs = sml.tile([1, QS], F32R, name=nm + "_rs", tag="ln_rs")
                with nc.allow_low_precision(reason="f32r rstd"):
                    nc.vector.reciprocal(out=rs, in_=sd)
                nc.tensor.matmul(mr_ps[:, 0, :], lhsT=ones_row,
                                 rhs=murow, start=True, stop=True)
                zc = finp.tile([D, QS], F32R, name=nm + "_zc", tag="ln_zc")
                nc.vector.tensor_sub(out=zc, in0=src, in1=mr_ps[:, 0, :])
                nc.tensor.matmul(mr_ps[:, 1, :], lhsT=ones_row,
                                 rhs=rs, start=True, stop=True)
                zm = finp.tile([D, QS], F32R, name=nm + "_zm", tag="ln_zm")
                nc.vector.tensor_mul(out=zm, in0=zc, in1=mr_ps[:, 1, :])
                dst = finp.tile([D, QS], dst_dt, name=nm, tag="ln_dst")
                nc.vector.tensor_scalar(out=dst, in0=zm, scalar1=gain,
                                        scalar2=bias_, op0=MULT, op1=ADD)
                return dst

            zhat = feat_ln(zt, preg_t, preb_t, BF16, "zhat")

            h_ps = fps.tile([D, 2, QS], F32, name="h_ps")
            for f in range(2):
                nc.tensor.matmul(h_ps[:, f, :],
                                 lhsT=w1_t[:, f * D:(f + 1) * D],
                                 rhs=zhat, start=True, stop=True)
            gel = finp.tile([D, 2, QS], BF16, name="gel")
            for f in range(2):
                nc.scalar.activation(out=gel[:, f, :], in_=h_ps[:, f, :],
                                     func=GELU, bias=b1_t[:, f:f + 1],
                                     scale=1.0)
            o2_ps = fps.tile([D, QS], F32, name="o2_ps", tag="zo")
            for f in range(2):
                nc.tensor.matmul(o2_ps, lhsT=w2_t[:, f, :], rhs=gel[:, f, :],
                                 start=(f == 0), stop=(f == 1))
            res = finp.tile([D, QS], F32R, name="res")
            nc.vector.tensor_scalar_add(out=res, in0=o2_ps, scalar1=b2_t)
            nc.vector.tensor_add(out=res, in0=res, in1=zhat)

            final = feat_ln(res, postg_t, postb_t, F32, "final")
            nc.sync.dma_start(out=out, in_=final)
            fpool.__exit__(None, None, None)
            accpool.__exit__(None, None, None)

    if split:
        _split_sync_waits(nc)
    return nc


# ---------------------------------------------------------------------------
def _prep_core_inputs(b, r, q, k, v, skip, q_ln_g, q_ln_b, Wq, bq, k_ln_g,
                      k_ln_b, Wk, bk, v_ln_g, v_ln_b, Wv, bv, Wp, bp,
                      pre_g, pre_b, W1, b1, W2, b2, post_g, post_b):
    f32 = np.float32
    bf16 = ml_dtypes.bfloat16

    def fold(W, g):
        wg = g[:, None] * W
        return (wg - wg.sum(0, keepdims=True) / 128.0).astype(f32)

    wq_all = SCALE * fold(Wq, q_ln_g)                      # [D, 128]
    wk_all = fold(Wk, k_ln_g)                              # [D, 128]
    wv_f = fold(Wv, v_ln_g)                                # [D, 128]
    wv_ext = np.zeros((D, HEADS, 33), f32)
    wv_ext[:, :, 0:32] = wv_f.reshape(D, HEADS, DH)
    wcat = np.concatenate([
        wq_all, wk_all, wv_ext.reshape(D, HEADS * 33),
        W1.astype(f32),
        W2.reshape(2, D, D).transpose(1, 0, 2).reshape(D, 2 * D),
    ], axis=1).astype(bf16)

    # bias folding: q_ln_b -> wbq (added to q-heads); v_ln_b -> bp'
    # (rides through attention as a constant, then Wp); k_ln_b drops out
    # (adds a per-query constant to all logits -> softmax invariant).
    wbq = (SCALE * (Wq.T @ q_ln_b)).astype(f32)            # [128]
    wbv = Wv.T @ v_ln_b                                    # [128]
    bpp = bp + Wp.T @ wbv                                  # [D]
    pad = np.zeros(64, f32)
    fcon = np.stack([
        np.concatenate([wbq[0:64], pad]),
        np.concatenate([wbq[64:128], pad]),
        bpp, b1[0:D], b1[D:2 * D], b2,
        pre_g, pre_b, post_g, post_b,
    ], axis=1).astype(f32)

    sl = slice(r * QS, (r + 1) * QS)
    return {
        "xq": np.ascontiguousarray(
            q[b].reshape(NCAM, D, Q)[:, :, sl]).astype(bf16),
        "xk": np.ascontiguousarray(k[b].reshape(NCAM, D, KC)).astype(bf16),
        "xv": np.ascontiguousarray(v[b].reshape(NCAM, D, KC)).astype(bf16),
        "wcat": wcat,
        "wp": np.ascontiguousarray(
            Wp.reshape(HEADS, DH, D).transpose(1, 0, 2), f32),
        "fcon": fcon,
        "skipb": np.ascontiguousarray(skip[b].reshape(D, Q)[:, sl], f32),
    }


def kernel(**inputs):
    if "nc" not in _cached:
        _cached["nc"] = _build_program()
    nc = _cached["nc"]
    args = {kk: np.asarray(vv) for kk, vv in inputs.items()}
    in_maps = [_prep_core_inputs(c // 4, c % 4, **args) for c in range(N_CORES)]
    res = run_bass_kernel_spmd(nc, in_maps, core_ids=list(range(N_CORES)))
    full = np.zeros((B, D, Q), np.float32)
    for c in range(N_CORES):
        b, r = c // 4, c % 4
        full[b][:, r * QS:(r + 1) * QS] = res.results[c]["out"]
    return full.reshape(B, D, 32, 32)


# revision 20
# speedup vs baseline: 1.1214x; 1.0461x over previous
"""BEV cross-attention kernel for Trainium2, 8-core SPMD.

Shard: core c handles (batch b=c//4, query slice r=c%4 of 256 BEV queries),
computing ALL 4 heads for its queries. Keys/values (6 cams x 1680) are
replicated per core. No collectives: each core's output is a disjoint
[D, 256] token slice; the host concatenates.

Layout: feature-major ("S^T") attention - scores [keys=120p, (head, q)=1024f]
so softmax exp runs on ScalarE with per-partition (per-key) scale=rstd_k and
bias=ln(rstd_v) (K/V LayerNorms folded through the exp; shared by all heads).
LN means fold into centered projection weights host-side; the softmax
denominator rides the PV matmul as a per-head ones column of V. No max
subtraction (logits are small by construction).

Engine budget: ScalarE does the 84 exps (the wall, ~88us); PE does all
projections + QK/PV in bf16 (1 cyc/col); DVE does squares (bf16 2x) and
evacs; Pool does V evacs, cross-partition reduces and broadcasts. Per-token
LN stats are produced token-major directly by 1-col PE matmuls against a
ones vector (no DRAM bounces anywhere).
"""
import numpy as np
import ml_dtypes

import concourse.bass as bass
import concourse.bass_isa as bass_isa
import concourse.mybir as mybir
import concourse.tile as tile
from concourse.bass_utils import run_bass_kernel_spmd

F32 = mybir.dt.float32
F32R = mybir.dt.float32r
BF16 = mybir.dt.bfloat16

HEADS, DH, D = 4, 32, 128
B, NCAM = 2, 6
Q = 32 * 32            # 1024 BEV queries per batch
QS = Q // 4            # 256 queries per core
KC = 28 * 60           # 1680 keys per camera
CW = 120               # key chunk width: 1680 = 14 * 120, no tail
NKCH = KC // CW        # 14
N_CORES = 8
EPS = 1e-5
SCALE = DH ** -0.5

_cached = {}


# ---------------------------------------------------------------------------
# walrus compat: this container's walrus rejects instructions carrying more
# than one semaphore wait; move excess waits onto same-engine NoOps.
_COMPUTE_ENGINES = None
_nopctr = [0]


def _split_sync_waits(nc, limit=1):
    global _COMPUTE_ENGINES
    if _COMPUTE_ENGINES is None:
        _COMPUTE_ENGINES = {
            mybir.EngineType.PE, mybir.EngineType.Activation,
            mybir.EngineType.Pool, mybir.EngineType.DVE, mybir.EngineType.SP,
        }
    for f in nc.m.functions:
        for bb in f.blocks:
            out, changed = [], False
            for inst in bb.instructions:
                si = inst.sync_info
                if (si is not None and len(si.on_wait) > limit
                        and inst.engine in _COMPUTE_ENGINES):
                    waits = list(si.on_wait)
                    n_extra = len(waits) - limit
                    for i in range(0, n_extra, limit):
                        nop = mybir.InstNoOp(name=f"wait-split-{_nopctr[0]}")
                        _nopctr[0] += 1
                        nop.engine = inst.engine
                        nop.sync_info = mybir.SyncInfo(
                            on_wait=waits[i:min(i + limit, n_extra)], on_update=[])
                        out.append(nop)
                    si.on_wait = waits[n_extra:]
                    changed = True
                out.append(inst)
            if changed:
                bb.instructions = out


# ---------------------------------------------------------------------------
def _build_program(split=True, collective=True, n_dev=N_CORES):
    nc = bass.Bass("TRN2", target_bir_lowering=False, debug=False,
                   num_devices=n_dev)

    xq = nc.dram_tensor("xq", [NCAM, D, QS], BF16, kind="ExternalInput").ap()
    xk = nc.dram_tensor("xk", [NCAM, D, KC], BF16, kind="ExternalInput").ap()
    xv = nc.dram_tensor("xv", [NCAM, D, KC], BF16, kind="ExternalInput").ap()
    # packed bf16 weights: [wq 128 | wk 128 | wv_ext 132 | w1 256 | w2 256]
    wcat = nc.dram_tensor("wcat", [D, 900], BF16, kind="ExternalInput").ap()
    wp = nc.dram_tensor("wp", [DH, HEADS, D], F32R, kind="ExternalInput").ap()
    # packed f32 per-feature consts:
    # [wbq_pairA, wbq_pairB, bp', b1_0, b1_1, b2, pre_g, pre_b, post_g,
    #  post_b] (wbq pair columns hold heads 0-1 / 2-3 in partitions 0..63)
    fcon = nc.dram_tensor("fcon", [D, 10], F32, kind="ExternalInput").ap()
    skipb = nc.dram_tensor("skipb", [D, QS], F32, kind="ExternalInput").ap()

    out = nc.dram_tensor("out", [D, QS], F32, kind="ExternalOutput").ap()

    EXP = mybir.ActivationFunctionType.Exp
    LN_ = mybir.ActivationFunctionType.Ln
    SQRT = mybir.ActivationFunctionType.Sqrt
    GELU = mybir.ActivationFunctionType.Gelu
    ADD = mybir.AluOpType.add
    MULT = mybir.AluOpType.mult

    with tile.TileContext(nc) as tc:
        with tc.tile_pool(name="consts", bufs=1) as consts, \
             tc.tile_pool(name="loads", bufs=3) as loads, \
             tc.tile_pool(name="sq", bufs=2) as sqp, \
             tc.tile_pool(name="kv", bufs=3) as kvp, \
             tc.tile_pool(name="sml", bufs=2) as sml, \
             tc.tile_pool(name="ee", bufs=3) as eep, \
             tc.tile_pool(name="fin", bufs=1) as finp:

            # ---- constant tiles (DMAs are issued after the first
            # camera loads, in the schedule section) ----
            wcat_t = consts.tile([D, 900], BF16, name="wcat_t")
            wq_t = wcat_t[:, 0:128]
            wk_t = wcat_t[:, 128:256]
            wv_t = wcat_t[:, 256:388]          # [D, 4*33]
            w1_t = wcat_t[:, 388:644]
            w2_t = wcat_t[:, 644:900].rearrange("p (f d) -> p f d", f=2)
            wp_t = consts.tile([DH, HEADS, D], F32R, name="wp_t")
            fcon_t = consts.tile([D, 10], F32, name="fcon_t")
            wbq_ab = fcon_t[:, 0:2]
            bpp_t = fcon_t[:, 2:3]
            b1_t = fcon_t[:, 3:5]
            b2_t = fcon_t[:, 5:6]
            preg_t = fcon_t[:, 6:7]
            preb_t = fcon_t[:, 7:8]
            postg_t = fcon_t[:, 8:9]
            postb_t = fcon_t[:, 9:10]
            skip_t = consts.tile([D, QS], F32, name="skip_t")

            eps_t = consts.tile([D, 1], F32, name="eps_t")
            nc.vector.memset(eps_t, EPS)
            onesb = consts.tile([D, 2], BF16, name="onesb")  # [1 | 1/128]
            nc.vector.memset(onesb[:, 0:1], 1.0)
            nc.vector.memset(onesb[:, 1:2], 1.0 / 128.0)
            onesr_f = consts.tile([D, 2], F32, name="onesr")  # [1 | 1/128]
            nc.vector.memset(onesr_f[:, 0:1], 1.0)
            nc.vector.memset(onesr_f[:, 1:2], 1.0 / 128.0)
            onesr = onesr_f.bitcast(F32R)
            ones_row_f = consts.tile([1, D], F32, name="ones_row")
            nc.vector.memset(ones_row_f, 1.0)
            ones_row = ones_row_f.bitcast(F32R)

            # ---- PSUM pools ----
            # banks: avt 2 + sc 2x2 + kp 1 + shared proj 1 = 8
            accpool = tc.tile_pool(name="accp", bufs=1, space="PSUM")
            accp = accpool.__enter__()
            scpool = tc.tile_pool(name="scp", bufs=2, space="PSUM")
            scp = scpool.__enter__()
            projpool = tc.tile_pool(name="projp", bufs=1, space="PSUM")
            projp = projpool.__enter__()

            avt = accp.tile([33, HEADS, QS], F32, name="avt")      # 2 banks
            # bank A: kproj ping-pong [0:210|210:420] | stats [420:476]
            ka_ps = projp.tile([D, 512], F32, name="ka_ps")        # 1 bank
            kp_ps = [ka_ps[:, 0:210], ka_ps[:, 210:420]]
            st_ps = ka_ps[0:CW, 420:476].rearrange("p (j c) -> p j c", j=4)
            # bank B: vproj ping-pong [0:132|132:264] | qproj [256:512]
            sh_ps = projp.tile([D, 512], F32, name="sh_ps")        # 1 bank
            vp_ps = [sh_ps[0:CW, 0:132], sh_ps[0:CW, 132:264]]
            qp_ps = [sh_ps[0:64, 0:256], sh_ps[0:64, 256:512]]

            # ---- per-camera phase 1, split so the ScalarE ops (finish)
            # can be emitted mid-attention of the previous camera and never
            # block the exp stream ----
            def load(n):
                xk_t = loads.tile([D, KC], BF16, name="xk_t", tag="xk")
                nc.sync.dma_start(out=xk_t, in_=xk[n])
                xv_t = loads.tile([D, KC], BF16, name="xv_t", tag="xv")
                nc.sync.dma_start(out=xv_t, in_=xv[n])
                xq_t = loads.tile([D, QS], BF16, name="xq_t", tag="xq")
                nc.sync.dma_start(out=xq_t, in_=xq[n])
                return xk_t, xv_t, xq_t

            def produce_a(n, ld):
                xk_t, xv_t, xq_t = ld

                x2k = sqp.tile([D, KC], BF16, name="x2k", tag="x2k")
                nc.vector.tensor_mul(out=x2k, in0=xk_t, in1=xk_t)
                x2v = sqp.tile([D, KC], BF16, name="x2v", tag="x2v")
                nc.vector.tensor_mul(out=x2v, in0=xv_t, in1=xv_t)
                x2q = sqp.tile([D, QS], BF16, name="x2q", tag="x2q")
                nc.vector.tensor_mul(out=x2q, in0=xq_t, in1=xq_t)

                # token-major stats via 1-col matmuls:
                # st rows: 0=k-mean, 1=v-mean, 2=k-sumsq, 3=v-sumsq
                for c in range(NKCH):
                    xkc = xk_t[:, c * CW:(c + 1) * CW]
                    xvc = xv_t[:, c * CW:(c + 1) * CW]
                    x2kc = x2k[:, c * CW:(c + 1) * CW]
                    x2vc = x2v[:, c * CW:(c + 1) * CW]
                    nc.tensor.matmul(st_ps[:, 0, c:c + 1], lhsT=xkc,
                                     rhs=onesb[:, 1:2], start=True, stop=True)
                    nc.tensor.matmul(st_ps[:, 1, c:c + 1], lhsT=xvc,
                                     rhs=onesb[:, 1:2], start=True, stop=True)
                    nc.tensor.matmul(st_ps[:, 2, c:c + 1], lhsT=x2kc,
                                     rhs=onesb[:, 0:1], start=True, stop=True)
                    nc.tensor.matmul(st_ps[:, 3, c:c + 1], lhsT=x2vc,
                                     rhs=onesb[:, 0:1], start=True, stop=True)

                st_sb = sml.tile([CW, 4, NKCH], F32, name="st_sb", tag="st")
                nc.vector.tensor_copy(out=st_sb, in_=st_ps)
                mu2 = sml.tile([CW, 2, NKCH], F32, name="mu2", tag="mu2")
                nc.vector.tensor_mul(out=mu2, in0=st_sb[:, 0:2, :],
                                     in1=st_sb[:, 0:2, :])
                var2 = sml.tile([CW, 2, NKCH], F32, name="var2", tag="var2")
                nc.vector.tensor_scalar_mul(out=var2, in0=st_sb[:, 2:4, :],
                                            scalar1=1.0 / 128.0)
                nc.vector.tensor_sub(out=var2, in0=var2, in1=mu2)

                # q stats rows (Pool C-reduce, off the PE/Act path)
                musum = sml.tile([1, QS], F32, name="musum", tag="musum")
                nc.gpsimd.tensor_reduce(out=musum, in_=xq_t,
                                        axis=mybir.AxisListType.C, op=ADD)
                sssum = sml.tile([1, QS], F32, name="sssum", tag="sssum")
                nc.gpsimd.tensor_reduce(out=sssum, in_=x2q,
                                        axis=mybir.AxisListType.C, op=ADD)
                muq = sml.tile([1, QS], F32, name="muq", tag="muq")
                nc.vector.tensor_scalar_mul(out=muq, in0=musum,
                                            scalar1=1.0 / 128.0)
                mu2q = sml.tile([1, QS], F32, name="mu2q", tag="mu2q")
                nc.vector.tensor_mul(out=mu2q, in0=muq, in1=muq)
                varq = sml.tile([1, QS], F32, name="varq", tag="varq")
                nc.vector.tensor_scalar_mul(out=varq, in0=sssum,
                                            scalar1=1.0 / 128.0)
                nc.vector.tensor_sub(out=varq, in0=varq, in1=mu2q)

                return xk_t, xv_t, xq_t, var2, varq

            def produce_b(n, pa):
                xk_t, xv_t, xq_t, var2, varq = pa
                # K projection, feature-major, one full-width matmul
                # per 210-chunk. Head pair A (partitions 0:63) is evacuated
                # in place into the stage tile; pair B (64:127) is shifted
                # down to its own base-0 tile with an SBUF->SBUF DMA, since
                # compute engines cannot move data across partitions.
                stage = kvp.tile([D, KC], BF16, name="khTs", tag="khTs")
                khT_b = kvp.tile([64, KC], BF16, name="khTb", tag="khTb")
                khT = [stage, khT_b]
                for j in range(8):
                    kp = kp_ps[j % 2]
                    nc.tensor.matmul(
                        kp, lhsT=wk_t,
                        rhs=xk_t[:, j * 210:(j + 1) * 210],
                        start=True, stop=True)
                    nc.vector.tensor_copy(
                        out=stage[:, j * 210:(j + 1) * 210], in_=kp)
                for g in range(2):
                    nc.sync.dma_start(
                        out=khT_b[:, g * 840:(g + 1) * 840],
                        in_=stage[64:128, g * 840:(g + 1) * 840])

                # V projection, token-major [120, 4, 33] per chunk
                # (col 32 of each head block is 0 from wv_ext; memset to 1
                # afterwards: softmax denominator ride-along)
                vhE = kvp.tile([CW, NKCH, HEADS, 33], BF16, name="vhE",
                               tag="vhE")
                nc.gpsimd.memset(vhE[:, :, :, 32], 1.0)
                vpair = sh_ps[0:CW, 0:264].rearrange(
                    "p (c h d) -> p c h d", c=2, h=4)
                for c in range(NKCH):
                    xvc = xv_t[:, c * CW:(c + 1) * CW]
                    vp = vp_ps[c % 2]
                    nc.tensor.matmul(vp, lhsT=xvc, rhs=wv_t,
                                     start=True, stop=True)
                    if c % 2 == 1:
                        nc.vector.tensor_copy(
                            out=vhE[:, c - 1:c + 1, :, 0:32],
                            in_=vpair[:, :, :, 0:32])
                return khT, vhE

            def produce(n, ld):
                pa = produce_a(n, ld)
                khT, vhE = produce_b(n, pa)
                return pa[2], pa[3], pa[4], khT, vhE

            def finish(n, prod):
                xq_t, var2, varq, khT, vhE = prod
                # ln(var+eps) for both K and V in one ScalarE op
                lnb = sml.tile([CW, 2, NKCH], F32, name="lnb", tag="lnb")
                nc.scalar.activation(out=lnb, in_=var2, func=LN_,
                                     bias=eps_t[0:CW, :], scale=1.0)
                rstdk = kvp.tile([CW, NKCH], F32, name="rstdk", tag="rstdk")
                nc.scalar.activation(out=rstdk, in_=lnb[:, 0, :], func=EXP,
                                     bias=0.0, scale=-0.5)
                lnrv = kvp.tile([CW, NKCH], F32, name="lnrv", tag="lnrv")
                nc.vector.tensor_scalar_mul(out=lnrv, in0=lnb[:, 1, :],
                                            scalar1=-0.5)

                sdq = sml.tile([1, QS], F32, name="sdq", tag="sdq")
                nc.scalar.activation(out=sdq, in_=varq, func=SQRT,
                                     bias=eps_t[0:1, :], scale=1.0)
                rqrow = sml.tile([1, QS], F32R, name="rqrow", tag="rqrow")
                with nc.allow_low_precision(reason="f32r rstd_q"):
                    nc.vector.reciprocal(out=rqrow, in_=sdq)
                rqbc = sh_ps[:, 0:256]
                nc.tensor.matmul(rqbc, lhsT=ones_row, rhs=rqrow,
                                 start=True, stop=True)
                # pre-scale x by rstd_q (commutes with the centered
                # projection), so the q matmuls have no ScalarE dependency
                xqn = sml.tile([D, QS], BF16, name="xqn", tag="xqn")
                nc.vector.tensor_mul(out=xqn, in0=xq_t, in1=rqbc)
                qhT = [kvp.tile([64, QS], BF16, name=f"qhT{p}",
                                tag=f"qhT{p}") for p in range(2)]
                for p in range(2):
                    nc.tensor.matmul(qp_ps[p],
                                     lhsT=wq_t[:, p * 64:(p + 1) * 64],
                                     rhs=xqn, start=True, stop=True)
                    nc.vector.tensor_scalar_add(
                        out=qhT[p], in0=qp_ps[p],
                        scalar1=wbq_ab[0:64, p:p + 1])
                return khT, vhE, rstdk, lnrv, qhT

            # ---- attention for one camera (chunk range) ----
            def attention(n, cam, c0, c1):
                khT, vhE, rstdk, lnrv, qhT = cam
                for c in range(c0, c1):
                    sc_ps = scp.tile([CW, HEADS, QS], F32, name="sc_ps",
                                     tag="sc")
                    for h in range(HEADS):
                        p, hh = divmod(h, 2)
                        nc.tensor.matmul(
                            sc_ps[:, h, :],
                            lhsT=khT[p][hh * DH:(hh + 1) * DH,
                                        c * CW:(c + 1) * CW],
                            rhs=qhT[p][hh * DH:(hh + 1) * DH, :],
                            start=True, stop=True)
                    et = eep.tile([CW, HEADS, QS], BF16, name="et", tag="et")
                    nc.scalar.activation(out=et, in_=sc_ps, func=EXP,
                                         bias=lnrv[:, c:c + 1],
                                         scale=rstdk[:, c:c + 1])
                    first = (n == 0 and c == 0)
                    last = (n == NCAM - 1 and c == NKCH - 1)
                    for h in range(HEADS):
                        nc.tensor.matmul(
                            avt[:, h, :],
                            lhsT=vhE[:, c, h, 0:33],
                            rhs=et[:, h, :],
                            start=first, stop=last)

            # ---- pipelined schedule: produce(n+1) and finish(n+1) are
            # emitted around the first half of attention(n) so no engine's
            # in-order queue ever blocks the exp stream ----
            cams = [None] * NCAM
            prods = [None] * NCAM
            ld0 = load(0)
            nc.sync.dma_start(out=wcat_t, in_=wcat)
            nc.sync.dma_start(out=fcon_t, in_=fcon)
            ld1 = load(1)
            nc.sync.dma_start(out=wp_t, in_=wp)
            nc.sync.dma_start(out=skip_t, in_=skipb)
            pa0 = produce_a(0, ld0)
            fin0 = finish(0, (pa0[2], pa0[3], pa0[4], None, None))
            khT0, vhE0 = produce_b(0, pa0)
            cams[0] = (khT0, vhE0, fin0[2], fin0[3], fin0[4])
            attention(0, cams[0], 0, 3)
            prods[1] = produce(1, ld1)
            for n in range(NCAM):
                attention(n, cams[n], 3 if n == 0 else 0, 4)
                if n + 1 < NCAM:
                    cams[n + 1] = finish(n + 1, prods[n + 1])
                attention(n, cams[n], 4, 9)
                if n + 2 < NCAM:
                    prods[n + 2] = produce(n + 2, load(n + 2))
                attention(n, cams[n], 9, NKCH)

            # ---- tail: normalize, project, skip, LN, MLP, LN ----
            rden = finp.tile([1, HEADS, QS], F32R, name="rden")
            with nc.allow_low_precision(reason="f32r denominator"):
                nc.vector.reciprocal(out=rden, in_=avt[32:33, :, :])

            projpool.__exit__(None, None, None)
            scpool.__exit__(None, None, None)
            fpool = tc.tile_pool(name="fps", bufs=1, space="PSUM")
            fps = fpool.__enter__()
            rd_ps = fps.tile([DH, 512], F32, name="rd_ps")
            avt_sb = finp.tile([DH, HEADS, QS], F32, name="avt_sb")
            nc.vector.tensor_copy(out=avt_sb, in_=avt[0:32, :, :])
            anorm = finp.tile([DH, HEADS, QS], F32R, name="anorm")
            rden_f = rden.rearrange("p h q -> p (h q)")
            anorm_f = anorm.rearrange("p h q -> p (h q)")
            avt_f = avt_sb.rearrange("p h q -> p (h q)")
            for j in range(2):
                nc.tensor.matmul(rd_ps, lhsT=ones_row[:, 0:DH],
                                 rhs=rden_f[:, j * 512:(j + 1) * 512],
                                 start=True, stop=True)
                nc.vector.tensor_mul(out=anorm_f[:, j * 512:(j + 1) * 512],
                                     in0=avt_f[:, j * 512:(j + 1) * 512],
                                     in1=rd_ps)

            zo_ps = fps.tile([D, QS], F32, name="zo_ps", tag="zo")
            for h in range(HEADS):
                nc.tensor.matmul(zo_ps, lhsT=wp_t[:, h, :],
                                 rhs=anorm[:, h, :],
                                 start=(h == 0), stop=(h == HEADS - 1))
            zt = finp.tile([D, QS], F32R, name="zt")
            nc.vector.tensor_add(out=zt, in0=zo_ps, in1=skip_t)
            nc.vector.tensor_scalar_add(out=zt, in0=zt, scalar1=bpp_t)

            row_ps = fps.tile([1, 2, QS], F32, name="row_ps")
            mr_ps = fps.tile([D, 2, QS], F32, name="mr_ps")

            def feat_ln(src, gain, bias_, dst_dt, nm):
                """LayerNorm across partitions (d) of src [128, QS].
                Critical path: s2 -> rows -> vr -> sub -> sqrt -> recip ->
                bcast -> zm -> gb; the mean copy/broadcast and zc run off
                the path on Pool/PE/DVE slack."""
                s2 = finp.tile([D, QS], F32R, name=nm + "_s2", tag="ln_s2")
                nc.vector.tensor_mul(out=s2, in0=src, in1=src)
                nc.tensor.matmul(row_ps[:, 0, :], lhsT=onesr[:, 1:2],
                                 rhs=src, start=True, stop=True)
                nc.tensor.matmul(row_ps[:, 1, :], lhsT=onesr[:, 0:1],
                                 rhs=s2, start=True, stop=True)
                murow = sml.tile([1, QS], F32R, name=nm + "_mu", tag="ln_mu")
                nc.scalar.copy(out=murow, in_=row_ps[:, 0, :])
                m2 = sml.tile([1, QS], F32, name=nm + "_m2", tag="ln_m2")
                nc.scalar.square(out=m2, in_=row_ps[:, 0, :])
                vr = sml.tile([1, QS], F32, name=nm + "_vr", tag="ln_vr")
                nc.vector.tensor_scalar_mul(out=vr, in0=row_ps[:, 1, :],
                                            scalar1=1.0 / 128.0)
                nc.vector.tensor_sub(out=vr, in0=vr, in1=m2)
                sd = sml.tile([1, QS], F32, name=nm + "_sd", tag="ln_sd")
                nc.scalar.activation(out=sd, in_=vr, func=SQRT,
                                     bias=eps_t[0:1, :], scale=1.0)
                rs = sml.tile([1, QS], F32R, name=nm + "_rs", tag="ln_rs")
                with nc.allow_low_precision(reason="f32r rstd"):
                    nc.vector.reciprocal(out=rs, in_=sd)
                nc.tensor.matmul(mr_ps[:, 0, :], lhsT=ones_row,
                                 rhs=murow, start=True, stop=True)
                zc = finp.tile([D, QS], F32R, name=nm + "_zc", tag="ln_zc")
                nc.vector.tensor_sub(out=zc, in0=src, in1=mr_ps[:, 0, :])
                nc.tensor.matmul(mr_ps[:, 1, :], lhsT=ones_row,
                                 rhs=rs, start=True, stop=True)
                zm = finp.tile([D, QS], F32R, name=nm + "_zm", tag="ln_zm")
                nc.vector.tensor_mul(out=zm, in0=zc, in1=mr_ps[:, 1, :])
                dst = finp.tile([D, QS], dst_dt, name=nm, tag="ln_dst")
                nc.vector.tensor_scalar(out=dst, in0=zm, scalar1=gain,
                                        scalar2=bias_, op0=MULT, op1=ADD)
                return dst

            zhat = feat_ln(zt, preg_t, preb_t, BF16, "zhat")

            h_ps = fps.tile([D, 2, QS], F32, name="h_ps")
            for f in range(2):
                nc.tensor.matmul(h_ps[:, f, :],
                                 lhsT=w1_t[:, f * D:(f + 1) * D],
                                 rhs=zhat, start=True, stop=True)
            gel = finp.tile([D, 2, QS], BF16, name="gel")
            for f in range(2):
                nc.scalar.activation(out=gel[:, f, :], in_=h_ps[:, f, :],
                                     func=GELU, bias=b1_t[:, f:f + 1],
                                     scale=1.0)
            o2_ps = fps.tile([D, QS], F32, name="o2_ps", tag="zo")
            for f in range(2):
                nc.tensor.matmul(o2_ps, lhsT=w2_t[:, f, :], rhs=gel[:, f, :],
                                 start=(f == 0), stop=(f == 1))
            res = finp.tile([D, QS], F32R, name="res")
            nc.vector.tensor_scalar_add(out=res, in0=o2_ps, scalar1=b2_t)
            nc.vector.tensor_add(out=res, in0=res, in1=zhat)

            final = feat_ln(res, postg_t, postb_t, F32, "final")
            nc.sync.dma_start(out=out, in_=final)
            fpool.__exit__(None, None, None)
            accpool.__exit__(None, None, None)

    if split:
        _split_sync_waits(nc)
    return nc


# ---------------------------------------------------------------------------
def _prep_core_inputs(b, r, q, k, v, skip, q_ln_g, q_ln_b, Wq, bq, k_ln_g,
                      k_ln_b, Wk, bk, v_ln_g, v_ln_b, Wv, bv, Wp, bp,
                      pre_g, pre_b, W1, b1, W2, b2, post_g, post_b):
    f32 = np.float32
    bf16 = ml_dtypes.bfloat16

    def fold(W, g):
        wg = g[:, None] * W
        return (wg - wg.sum(0, keepdims=True) / 128.0).astype(f32)

    wq_all = SCALE * fold(Wq, q_ln_g)                      # [D, 128]
    wk_all = fold(Wk, k_ln_g)                              # [D, 128]
    wv_f = fold(Wv, v_ln_g)                                # [D, 128]
    wv_ext = np.zeros((D, HEADS, 33), f32)
    wv_ext[:, :, 0:32] = wv_f.reshape(D, HEADS, DH)
    wcat = np.concatenate([
        wq_all, wk_all, wv_ext.reshape(D, HEADS * 33),
        W1.astype(f32),
        W2.reshape(2, D, D).transpose(1, 0, 2).reshape(D, 2 * D),
    ], axis=1).astype(bf16)

    # bias folding: q_ln_b -> wbq (added to q-heads); v_ln_b -> bp'
    # (rides through attention as a constant, then Wp); k_ln_b drops out
    # (adds a per-query constant to all logits -> softmax invariant).
    wbq = (SCALE * (Wq.T @ q_ln_b)).astype(f32)            # [128]
    wbv = Wv.T @ v_ln_b                                    # [128]
    bpp = bp + Wp.T @ wbv                                  # [D]
    pad = np.zeros(64, f32)
    fcon = np.stack([
        np.concatenate([wbq[0:64], pad]),
        np.concatenate([wbq[64:128], pad]),
        bpp, b1[0:D], b1[D:2 * D], b2,
        pre_g, pre_b, post_g, post_b,
    ], axis=1).astype(f32)

    sl = slice(r * QS, (r + 1) * QS)
    return {
        "xq": np.ascontiguousarray(
            q[b].reshape(NCAM, D, Q)[:, :, sl]).astype(bf16),
        "xk": np.ascontiguousarray(k[b].reshape(NCAM, D, KC)).astype(bf16),
        "xv": np.ascontiguousarray(v[b].reshape(NCAM, D, KC)).astype(bf16),
        "wcat": wcat,
        "wp": np.ascontiguousarray(
            Wp.reshape(HEADS, DH, D).transpose(1, 0, 2), f32),
        "fcon": fcon,
        "skipb": np.ascontiguousarray(skip[b].reshape(D, Q)[:, sl], f32),
    }


def kernel(**inputs):
    if "nc" not in _cached:
        _cached["nc"] = _build_program()
    nc = _cached["nc"]
    args = {kk: np.asarray(vv) for kk, vv in inputs.items()}
    in_maps = [_prep_core_inputs(c // 4, c % 4, **args) for c in range(N_CORES)]
    res = run_bass_kernel_spmd(nc, in_maps, core_ids=list(range(N_CORES)))
    full = np.zeros((B, D, Q), np.float32)
    for c in range(N_CORES):
        b, r = c // 4, c % 4
        full[b][:, r * QS:(r + 1) * QS] = res.results[c]["out"]
    return full.reshape(B, D, 32, 32)
